# revision 1
# baseline (speedup 1.0000x reference)
"""MoE (8 experts, top-2, shared expert) Trainium2 kernel.

Expert-parallel over 8 NeuronCores. The host performs only the dispatch
decision (top-2 expert ids -> compact per-expert token lists) and data
layout (transposes/gathers); all floating-point model math — router
logits, gates, expert SwiGLU, shared expert, and the cross-core combine
(ReduceScatter) — runs on device in fp32r matmuls with fp32 accumulation.

Device program per core (SPMD, identical program, per-core data):
  D1: hts[176, T] = silu(sw1_slice @ x) * (sw3_slice @ x)  (all tokens)
  A:  router logits for compact tokens (matmul) * validity mask -> gates
  B:  ht[I, C] = silu(w1 @ xg) * (w3 @ xg)   (compact tokens)
  C:  y[ct] = gate * (ht.T @ w2t) -> eacc[C, H] (dense write, compact order)
  D2: acc[t] = hts.T @ sw2_slice + eacc[inv_idx[t]]   (indirect GATHER with
      zero-row sentinel for tokens not routed to this core)
  E:  ReduceScatter(add) over acc -> this core's 256-token output slice
"""

import numpy as np

H = 1024          # hidden
I = 1408          # moe intermediate
E = 8             # experts == cores
T = 2048          # tokens (2*1024)
TOPK = 2
C = 640           # compact per-expert token capacity (max observed 540)
CH = 320          # ht token chunk (2 chunks; >=256 keeps fp32r at full rate)
ILOC = I // E     # 176: shared-expert intermediate slice per core
TSL = T // E      # 256: output token slice per core
KT = H // 128     # 8 contraction tiles over H
IT = I // 128     # 11 tiles over I
CT = C // 128     # 5 compact token tiles
TT = T // 128     # 16 token tiles
SIP = (128, ILOC - 128)   # shared I-slice partition tiles: 128 + 48
NCORES = 8
DTYPE = "f32r"     # "f32r" (full precision-ish) or "bf16" (faster DMA)

_BUILD_CACHE = {}


def _build(reps=1, use_cc=True, dtype=None, cap=None):
    lean = cap is not None and cap > C
    import concourse.bacc as bacc
    import concourse.bass as bass
    import concourse.mybir as mybir
    from concourse import tile
    from contextlib import ExitStack

    f32 = mybir.dt.float32
    f32r = mybir.dt.float32r
    i32 = mybir.dt.int32
    dt_mm = mybir.dt.bfloat16 if (dtype or DTYPE) == "bf16" else f32r
    AF = mybir.ActivationFunctionType
    MUL = mybir.AluOpType.mult

    C_ = cap or C
    CT_ = C_ // 128
    n_ch = max(1, (C_ + 511) // 512)
    CH_ = C_ // n_ch
    assert CH_ * n_ch == C_ and CH_ % 64 == 0, (C_, CH_)

    nc = bacc.Bacc("TRN2", target_bir_lowering=False, debug=False,
                   num_devices=NCORES)

    xg = nc.declare_dram_parameter("xg", [H, C_], f32r, isOutput=False)
    xt = nc.declare_dram_parameter("xt", [H, T], dt_mm, isOutput=False)
    w1t = nc.declare_dram_parameter("w1t", [IT, H, 128], dt_mm, isOutput=False)
    w3t = nc.declare_dram_parameter("w3t", [IT, H, 128], dt_mm, isOutput=False)
    w2t = nc.declare_dram_parameter("w2t", [I, H], dt_mm, isOutput=False)
    s1t = nc.declare_dram_parameter("s1t", [H, ILOC], dt_mm, isOutput=False)
    s3t = nc.declare_dram_parameter("s3t", [H, ILOC], dt_mm, isOutput=False)
    s2t = nc.declare_dram_parameter("s2t", [ILOC, H], dt_mm, isOutput=False)
    rwe = nc.declare_dram_parameter("rwe", [H, 16], f32r, isOutput=False)
    invi = nc.declare_dram_parameter("invi", [T, 1], i32, isOutput=False)
    msk = nc.declare_dram_parameter("msk", [128, CT_], f32, isOutput=False)
    out = nc.declare_dram_parameter("out", [TSL, H], f32, isOutput=True)

    acc = nc.dram_tensor("acc", [T, H], f32)
    eacc = nc.dram_tensor("eacc", [C_ + 128, H], f32)
    rs_out = nc.dram_tensor("rs_out", [TSL, H], f32)

    with tile.TileContext(nc) as tc, ExitStack() as ctx:
        sres = ctx.enter_context(tc.tile_pool(name="sres", bufs=1))
        wstr = ctx.enter_context(tc.tile_pool(name="wstr",
                                              bufs=1 if lean else 2))
        xstr = ctx.enter_context(tc.tile_pool(name="xstr",
                                              bufs=1 if lean else 2))
        work = ctx.enter_context(tc.tile_pool(name="work", bufs=2))
        psA = ctx.enter_context(tc.tile_pool(name="psA", bufs=2, space="PSUM"))
        psB = ctx.enter_context(tc.tile_pool(name="psB", bufs=2, space="PSUM"))
        psY = ctx.enter_context(tc.tile_pool(name="psY", bufs=4, space="PSUM"))

        TCH = 256
        for _rep in range(reps):
            # ---- resident loads ----
            s13_sb = sres.tile([128, 2 * KT * ILOC], dt_mm, tag="s13_sb",
                               name="s13_sb")
            for k in range(KT):
                nc.sync.dma_start(s13_sb[:, k * ILOC:(k + 1) * ILOC],
                                  s1t[k * 128:(k + 1) * 128, :])
                nc.sync.dma_start(
                    s13_sb[:, (KT + k) * ILOC:(KT + k + 1) * ILOC],
                    s3t[k * 128:(k + 1) * 128, :])
            xg_sb = sres.tile([128, KT * C_], f32r, tag="xg_sb", name="xg_sb")
            for k in range(KT):
                nc.sync.dma_start(xg_sb[:, k * C_:(k + 1) * C_],
                                  xg[k * 128:(k + 1) * 128, :])
            rwe_sb = sres.tile([128, KT * 16], f32r, tag="rwe_sb",
                               name="rwe_sb")
            if dt_mm is f32r:
                xgb_sb = xg_sb
            else:
                xgb_sb = sres.tile([128, KT * C_], dt_mm, tag="xgb_sb",
                                   name="xgb_sb")
                for k in range(KT):
                    nc.vector.tensor_copy(xgb_sb[:, k * C_:(k + 1) * C_],
                                          xg_sb[:, k * C_:(k + 1) * C_])
            nc.sync.dma_start(rwe_sb[:],
                              rwe.rearrange("(k p) o -> p k o", p=128))
            invi_sb = sres.tile([128, TT], i32, tag="invi_sb", name="invi_sb")
            nc.sync.dma_start(invi_sb[:],
                              invi.rearrange("(c p) o -> p c o", p=128))
            msk_sb = sres.tile([128, CT_], f32, tag="msk_sb", name="msk_sb")
            nc.sync.dma_start(msk_sb[:], msk[:, :])
            s2_sb = sres.tile([128, 2 * H], dt_mm, tag="s2_sb", name="s2_sb")
            nc.sync.dma_start(s2_sb[:, 0:H], s2t[0:128, :])
            nc.sync.dma_start(s2_sb[:SIP[1], H:2 * H], s2t[128:ILOC, :])
            # zero sentinel row block for the combine gather
            ztile = work.tile([128, H], f32, tag="ztile", name="ztile",
                              bufs=1)
            nc.gpsimd.memset(ztile[:], 0.0)
            nc.sync.dma_start(eacc[C_:C_ + 128, :], ztile[:])

            # ---- D1: shared-expert hts[176, T] over all tokens ----
            hts = sres.tile([128, 2 * T], dt_mm, tag="hts", name="hts")
            for tt in range(T // TCH):
                xc = xstr.tile([128, KT * TCH], dt_mm, tag="xc", name="xc")
                for k in range(KT):
                    nc.sync.dma_start(
                        xc[:, k * TCH:(k + 1) * TCH],
                        xt[k * 128:(k + 1) * 128, tt * TCH:(tt + 1) * TCH])
                for si in range(2):
                    sip = SIP[si]
                    psa = psA.tile([128, TCH], f32, tag="a", name="psa_s",
                                   space="PSUM")
                    psb = psB.tile([128, TCH], f32, tag="b", name="psb_s",
                                   space="PSUM")
                    for k in range(KT):
                        nc.tensor.matmul(
                            psa[:sip, :],
                            lhsT=s13_sb[:, k * ILOC + si * 128:
                                        k * ILOC + si * 128 + sip],
                            rhs=xc[:, k * TCH:(k + 1) * TCH],
                            start=(k == 0), stop=(k == KT - 1))
                    for k in range(KT):
                        nc.tensor.matmul(
                            psb[:sip, :],
                            lhsT=s13_sb[:, (KT + k) * ILOC + si * 128:
                                        (KT + k) * ILOC + si * 128 + sip],
                            rhs=xc[:, k * TCH:(k + 1) * TCH],
                            start=(k == 0), stop=(k == KT - 1))
                    sact = work.tile([128, TCH], f32, tag="sact_s",
                                     name="sact_s",
                                     bufs=1 if lean else None)
                    nc.scalar.activation(sact[:sip, :], psa[:sip, :], AF.Silu)
                    nc.vector.tensor_tensor(
                        out=hts[:sip, si * T + tt * TCH:
                                si * T + (tt + 1) * TCH],
                        in0=sact[:sip, :], in1=psb[:sip, :], op=MUL)

            # ---- Part A: logits for compact tokens -> gates ----
            gates_sb = sres.tile([128, CT_], f32, tag="gates_sb",
                                 name="gates_sb")
            for ct in range(CT_):
                psl = psY.tile([128, 512], f32, tag="y", name="psl",
                               space="PSUM")
                for k in range(KT):
                    nc.tensor.matmul(
                        psl[:, 0:16],
                        lhsT=xg_sb[:, k * C_ + ct * 128: k * C_ + (ct + 1) * 128],
                        rhs=rwe_sb[:, k * 16:(k + 1) * 16],
                        start=(k == 0), stop=(k == KT - 1))
                nc.vector.tensor_tensor(out=gates_sb[:, ct:ct + 1],
                                        in0=psl[:, 0:1],
                                        in1=msk_sb[:, ct:ct + 1], op=MUL)

            # ---- w2 resident load (overlaps with B's compute) ----
            w2_sb = sres.tile([128, IT * H], dt_mm, tag="w2_sb", name="w2_sb")
            for i in range(IT):
                nc.sync.dma_start(w2_sb[:, i * H:(i + 1) * H],
                                  w2t[i * 128:(i + 1) * 128, :])

            # ---- Part B: expert ht[I, C] = silu(w1@x) * (w3@x) ----
            ht_sb = sres.tile([128, IT * C_], dt_mm, tag="ht_sb", name="ht_sb")
            for i in range(IT):
                w1b = wstr.tile([128, KT * 128], dt_mm, tag="w1b", name="w1b")
                nc.sync.dma_start(w1b[:],
                                  w1t[i].rearrange("(k p) m -> p k m", p=128))
                w3b = wstr.tile([128, KT * 128], dt_mm, tag="w3b", name="w3b")
                nc.sync.dma_start(w3b[:],
                                  w3t[i].rearrange("(k p) m -> p k m", p=128))
                for cc in range(n_ch):
                    psa = psA.tile([128, CH_], f32, tag="a", name="psa",
                                   space="PSUM")
                    psb = psB.tile([128, CH_], f32, tag="b", name="psb",
                                   space="PSUM")
                    for k in range(KT):
                        nc.tensor.matmul(
                            psa[:],
                            lhsT=w1b[:, k * 128:(k + 1) * 128],
                            rhs=xgb_sb[:, k * C_ + cc * CH_: k * C_ + (cc + 1) * CH_],
                            start=(k == 0), stop=(k == KT - 1))
                    for k in range(KT):
                        nc.tensor.matmul(
                            psb[:],
                            lhsT=w3b[:, k * 128:(k + 1) * 128],
                            rhs=xgb_sb[:, k * C_ + cc * CH_: k * C_ + (cc + 1) * CH_],
                            start=(k == 0), stop=(k == KT - 1))
                    sact = work.tile([128, CH_], f32, tag="sact", name="sact")
                    nc.scalar.activation(sact[:], psa[:], AF.Silu)
                    nc.vector.tensor_tensor(
                        out=ht_sb[:, i * C_ + cc * CH_: i * C_ + (cc + 1) * CH_],
                        in0=sact[:], in1=psb[:], op=MUL)

            # ---- Part C: expert y (gated) -> eacc, dense compact order ----
            for ct in range(CT_):
                ysb_c = work.tile([128, H], f32, tag="ysb_c", name="ysb_c",
                                  bufs=1 if lean else None)
                for hh in range(2):
                    psy = psY.tile([128, 512], f32, tag="y", name="psy",
                                   space="PSUM")
                    for i in range(IT):
                        nc.tensor.matmul(
                            psy[:],
                            lhsT=ht_sb[:, i * C_ + ct * 128: i * C_ + (ct + 1) * 128],
                            rhs=w2_sb[:, i * H + hh * 512: i * H + hh * 512 + 512],
                            start=(i == 0), stop=(i == IT - 1))
                    nc.scalar.activation(
                        ysb_c[:, hh * 512:(hh + 1) * 512],
                        psy[:], AF.Copy, scale=gates_sb[:, ct:ct + 1])
                nc.sync.dma_start(eacc[ct * 128:(ct + 1) * 128, :], ysb_c[:])

            # ---- D2: acc[t] = hts.T @ sw2_slice + eacc[inv_idx[t]] ----
            for trow in range(TT):
                geacc = work.tile([128, H], f32, tag="geacc",
                                  name="geacc", bufs=2 if lean else 3)
                nc.gpsimd.indirect_dma_start(
                    out=geacc[:], out_offset=None,
                    in_=eacc[:, :],
                    in_offset=bass.IndirectOffsetOnAxis(
                        ap=invi_sb[:, trow:trow + 1], axis=0))
                ysb = work.tile([128, H], f32, tag="ysb", name="ysb",
                                bufs=2 if lean else 3)
                for hh in range(2):
                    psy = psY.tile([128, 512], f32, tag="y", name="psy_s",
                                   space="PSUM")
                    nc.tensor.matmul(
                        psy[:],
                        lhsT=hts[:, trow * 128:(trow + 1) * 128],
                        rhs=s2_sb[:, hh * 512:(hh + 1) * 512],
                        start=True, stop=False)
                    nc.tensor.matmul(
                        psy[:],
                        lhsT=hts[:SIP[1], T + trow * 128: T + (trow + 1) * 128],
                        rhs=s2_sb[:SIP[1], H + hh * 512: H + (hh + 1) * 512],
                        start=False, stop=True)
                    nc.vector.tensor_add(ysb[:, hh * 512:(hh + 1) * 512],
                                         psy[:],
                                         geacc[:, hh * 512:(hh + 1) * 512])
                nc.sync.dma_start(acc[trow * 128:(trow + 1) * 128, :],
                                  ysb[:])

            # ---- Part E: cross-core combine + output ----
            # (A 2-way split RS overlapped with D2's tail models WORSE:
            # 266.8us vs 263.2us — D2's tail is too short to hide a
            # collective and the extra launch overhead nets a loss.)
            if use_cc:
                nc.gpsimd.collective_compute(
                    "ReduceScatter",
                    mybir.AluOpType.add,
                    replica_groups=[list(range(NCORES))],
                    ins=[acc[:, :]],
                    outs=[rs_out[:, :]],
                )
                src_t = rs_out
            else:
                src_t = acc
            nc.sync.dma_start(out[:, :], src_t[0:TSL, :])

    nc.finalize()
    return nc


def _get_nc(reps=1):
    key = (reps, DTYPE, C)
    if key not in _BUILD_CACHE:
        _BUILD_CACHE[key] = _build(reps)
    return _BUILD_CACHE[key]


def _count_max(x2, router_w):
    logits = x2 @ router_w.T
    order = np.argsort(-logits, axis=1, kind="stable")[:, :TOPK]
    return max(int((order == e).any(axis=1).sum()) for e in range(E))


def _dispatch(x2, router_w, cap=None):
    """Host-side sharding decision: per-expert compact token lists."""
    cap = cap or C
    logits = x2 @ router_w.T                      # [T, E] fp32, dispatch only
    order = np.argsort(-logits, axis=1, kind="stable")[:, :TOPK]
    per_core = []
    all_rows = np.arange(T)
    for e in range(E):
        rows = all_rows[(order == e).any(axis=1)]
        ce = len(rows)
        assert ce <= cap, f"expert {e} overflow: {ce} > {cap}"
        unused = np.setdiff1d(all_rows, rows, assume_unique=True)
        pad = unused[:cap - ce]
        if len(pad) < cap - ce:   # cap > T - ce: reuse unused rows cyclically
            extra = np.resize(unused, cap - ce)
            pad = extra
        idx_full = np.concatenate([rows, pad]).astype(np.int32)
        mask = (np.arange(cap) < ce).astype(np.float32)
        inv = np.full(T, cap, dtype=np.int32)     # sentinel -> zero row
        inv[rows] = np.arange(ce, dtype=np.int32)
        per_core.append((idx_full, mask, inv))
    return per_core


def _make_in_maps(x2, router_w, w1, w2, w3, sw1, sw2, sw3, cap=None):
    if DTYPE == "bf16":
        import ml_dtypes
        np_mm = ml_dtypes.bfloat16
    else:
        np_mm = np.float32
    cap = cap or C
    dispatch = _dispatch(x2, router_w, cap)
    xt_host = np.ascontiguousarray(x2.T.astype(np_mm))
    in_maps = []
    for e in range(E):
        idx_full, mask, inv = dispatch[e]
        in_maps.append({
            "xg": np.ascontiguousarray(x2[idx_full].T),
            "xt": xt_host,
            "w1t": np.ascontiguousarray(
                np.asarray(w1[e], dtype=np.float32).reshape(IT, 128, H)
                .transpose(0, 2, 1).astype(np_mm)),
            "w3t": np.ascontiguousarray(
                np.asarray(w3[e], dtype=np.float32).reshape(IT, 128, H)
                .transpose(0, 2, 1).astype(np_mm)),
            "w2t": np.ascontiguousarray(
                np.asarray(w2[e], np.float32).T.astype(np_mm)),
            "s1t": np.ascontiguousarray(
                np.asarray(sw1[e * ILOC:(e + 1) * ILOC, :], np.float32)
                .T.astype(np_mm)),
            "s3t": np.ascontiguousarray(
                np.asarray(sw3[e * ILOC:(e + 1) * ILOC, :], np.float32)
                .T.astype(np_mm)),
            "s2t": np.ascontiguousarray(
                np.asarray(sw2[:, e * ILOC:(e + 1) * ILOC], np.float32)
                .T.astype(np_mm)),
            "rwe": np.ascontiguousarray(
                np.repeat(np.asarray(router_w[e], np.float32).reshape(H, 1),
                          16, axis=1)),
            "invi": inv.reshape(T, 1),
            "msk": np.ascontiguousarray(mask.reshape(cap // 128, 128).T),
        })
    return in_maps


def kernel(x, router_w, w1, w2, w3, sw1, sw2, sw3):
    from concourse.bass_utils import run_bass_kernel_spmd

    in_dtype = x.dtype
    x2 = np.ascontiguousarray(x.reshape(T, H), dtype=np.float32)
    router_w = np.asarray(router_w, dtype=np.float32)
    cap = C
    cmax = _count_max(x2, router_w)
    if cmax > C:   # unlikely re-routed inputs: rebuild with a larger capacity
        step = 256 if cmax <= 1024 else 512
        cap = -((-cmax) // step) * step
    key = (1, DTYPE, cap)
    if key not in _BUILD_CACHE:
        _BUILD_CACHE[key] = _build(1, cap=cap)
    nc = _BUILD_CACHE[key]

    in_maps = _make_in_maps(x2, router_w, w1, w2, w3, sw1, sw2, sw3, cap)
    res = run_bass_kernel_spmd(nc, in_maps, list(range(NCORES)))
    out = np.concatenate([res.results[i]["out"] for i in range(NCORES)],
                         axis=0)
    return out.reshape(x.shape).astype(in_dtype)



# revision 2
# speedup vs baseline: 1.4020x; 1.4020x over previous
"""MoE (8 experts, top-2, shared expert) Trainium2 kernel.

Expert-parallel over 8 NeuronCores, bf16 matmuls (fp32 PSUM accumulate).
The host performs only the dispatch decision (top-2 expert ids -> compact
per-expert token lists) and data layout (every tensor pre-arranged into its
exact SBUF image so each load is one large contiguous DMA); all model FP
math — router logits, gates, expert SwiGLU, shared expert, cross-core
combine — runs on device.

Device program per core (SPMD, identical program, per-core data):
  A:  router logits for compact tokens (matmul) * validity mask -> gates
  B:  ht[I, C] = silu(w1 @ xg) * (w3 @ xg)          (compact tokens)
  C:  y[ct] = gate * (ht.T @ w2) -> indirect-scatter rows into acc[token]
      (acc pre-zeroed during the load phase; unrouted tokens stay 0)
  RS: ReduceScatter(add, bf16) over acc -> rst (this core's 256 rows);
      carries ONLY expert contributions, so it overlaps with...
  S:  shared expert (full I) for this core's OWN 256 tokens -> hfin
  F:  out = rst + hfin   (bf16; host upcasts to f32)
"""

import numpy as np

H = 1024          # hidden
I = 1408          # moe intermediate
E = 8             # experts == cores
T = 2048          # tokens (2*1024)
TOPK = 2
C = 576           # compact per-expert token capacity (max observed 540)
ILOC = I // E     # 176 (unused by the kernel; kept for reference)
TSL = T // E      # 256: output token slice per core
KT = H // 128     # 8 contraction tiles over H
IT = I // 128     # 11 tiles over I
NCORES = 8

_BUILD_CACHE = {}


def _cap_geom(cap):
    """Token tiles (offset, len<=128) and B free-dim chunks (<=512)."""
    assert cap % 64 == 0, cap
    tiles = []
    off = 0
    while off < cap:
        L = min(128, cap - off)
        tiles.append((off, L))
        off += L
    chunks = []
    off = 0
    while off < cap:
        L = min(512, cap - off)
        chunks.append((off, L))
        off += L
    return tiles, chunks


def _build(reps=1, use_cc=True, dtype=None, cap=None):
    import concourse.bacc as bacc
    import concourse.bass as bass
    import concourse.mybir as mybir
    from concourse import tile
    from contextlib import ExitStack

    f32 = mybir.dt.float32
    bf16 = mybir.dt.bfloat16
    i32 = mybir.dt.int32
    AF = mybir.ActivationFunctionType
    MUL = mybir.AluOpType.mult

    cap = cap or C
    tiles, chunks = _cap_geom(cap)
    NT = len(tiles)

    nc = bacc.Bacc("TRN2", target_bir_lowering=False, debug=False,
                   num_devices=NCORES)

    # host-prepared SBUF images, one contiguous DMA each
    xgb = nc.declare_dram_parameter("xgb", [128, KT * cap], bf16,
                                    isOutput=False)
    w1i = nc.declare_dram_parameter("w1i", [128, IT * KT * 128], bf16,
                                    isOutput=False)
    w3i = nc.declare_dram_parameter("w3i", [128, IT * KT * 128], bf16,
                                    isOutput=False)
    w2i = nc.declare_dram_parameter("w2i", [128, IT * H], bf16,
                                    isOutput=False)
    s1i = nc.declare_dram_parameter("s1i", [128, IT * KT * 128], bf16,
                                    isOutput=False)
    s3i = nc.declare_dram_parameter("s3i", [128, IT * KT * 128], bf16,
                                    isOutput=False)
    s2i = nc.declare_dram_parameter("s2i", [128, IT * H], bf16,
                                    isOutput=False)
    xo = nc.declare_dram_parameter("xo", [128, KT * TSL], bf16,
                                   isOutput=False)
    rwe = nc.declare_dram_parameter("rwe", [128, KT * 16], bf16,
                                    isOutput=False)
    idx = nc.declare_dram_parameter("idx", [128, NT], i32, isOutput=False)
    msk = nc.declare_dram_parameter("msk", [128, NT], f32, isOutput=False)
    out = nc.declare_dram_parameter("out", [TSL, H], bf16, isOutput=True)

    acc = nc.dram_tensor("acc", [T, H], bf16)
    rst = nc.dram_tensor("rst", [TSL, H], bf16)

    with tile.TileContext(nc) as tc, ExitStack() as ctx:
        sres = ctx.enter_context(tc.tile_pool(name="sres", bufs=1))
        wbig = ctx.enter_context(tc.tile_pool(name="wbig", bufs=1))
        work = ctx.enter_context(tc.tile_pool(name="work", bufs=2))
        psA = ctx.enter_context(tc.tile_pool(name="psA", bufs=2, space="PSUM"))
        psB = ctx.enter_context(tc.tile_pool(name="psB", bufs=2, space="PSUM"))
        psY = ctx.enter_context(tc.tile_pool(name="psY", bufs=2, space="PSUM"))
        psL = ctx.enter_context(tc.tile_pool(name="psL", bufs=1, space="PSUM"))

        for _rep in range(reps):
            # ---- loads, in consumption order ----
            rwe_sb = sres.tile([128, KT * 16], bf16, tag="rwe_sb",
                               name="rwe_sb")
            nc.sync.dma_start(rwe_sb[:], rwe[:, :])
            idx_sb = sres.tile([128, NT], i32, tag="idx_sb", name="idx_sb")
            nc.sync.dma_start(idx_sb[:], idx[:, :])
            msk_sb = sres.tile([128, NT], f32, tag="msk_sb", name="msk_sb")
            nc.sync.dma_start(msk_sb[:], msk[:, :])
            xgb_sb = sres.tile([128, KT * cap], bf16, tag="xgb_sb",
                               name="xgb_sb")
            nc.sync.dma_start(xgb_sb[:], xgb[:, :])
            # expert weights: streamed per-i so B starts after ~0.5 MB
            w1_sb = wbig.tile([128, IT * KT * 128], bf16, tag="wa",
                              name="w1_sb")
            w3_sb = wbig.tile([128, IT * KT * 128], bf16, tag="wb",
                              name="w3_sb")
            for i in range(IT):
                sl = slice(i * KT * 128, (i + 1) * KT * 128)
                nc.sync.dma_start(w1_sb[:, sl], w1i[:, sl])
                nc.sync.dma_start(w3_sb[:, sl], w3i[:, sl])
            # zero-fill acc (4 DMAs; off the critical path)
            ztile = work.tile([128, 4 * H], bf16, tag="ztile", name="ztile",
                              bufs=1)
            nc.gpsimd.memset(ztile[:], 0.0)
            for z in range(T // 512):
                nc.sync.dma_start(
                    acc[z * 512:(z + 1) * 512, :]
                    .rearrange("(c p) h -> p c h", p=128),
                    ztile[:])
            w2_sb = wbig.tile([128, IT * H], bf16, tag="wc", name="w2_sb")
            nc.sync.dma_start(w2_sb[:], w2i[:, :])
            xo_sb = sres.tile([128, KT * TSL], bf16, tag="xo_sb",
                              name="xo_sb")
            nc.sync.dma_start(xo_sb[:], xo[:, :])

            gates_sb = sres.tile([128, NT], f32, tag="gates_sb",
                                 name="gates_sb")
            ht_sb = sres.tile([128, IT * cap], bf16, tag="ht_sb",
                              name="ht_sb")

            # ---- B: expert ht = silu(w1@xg)*(w3@xg); A after first i ----
            for i in range(IT):
                for (boff, BL) in chunks:
                    psa = psA.tile([128, 512], f32, tag="a", name="psa",
                                   space="PSUM")
                    psb = psB.tile([128, 512], f32, tag="b", name="psb",
                                   space="PSUM")
                    for k in range(KT):
                        nc.tensor.matmul(
                            psa[:, :BL],
                            lhsT=w1_sb[:, (i * KT + k) * 128:
                                       (i * KT + k + 1) * 128],
                            rhs=xgb_sb[:, k * cap + boff:k * cap + boff + BL],
                            start=(k == 0), stop=(k == KT - 1))
                    for k in range(KT):
                        nc.tensor.matmul(
                            psb[:, :BL],
                            lhsT=w3_sb[:, (i * KT + k) * 128:
                                       (i * KT + k + 1) * 128],
                            rhs=xgb_sb[:, k * cap + boff:k * cap + boff + BL],
                            start=(k == 0), stop=(k == KT - 1))
                    sact = work.tile([128, 512], f32, tag="sact",
                                     name="sact")
                    nc.scalar.activation(sact[:, :BL], psa[:, :BL], AF.Silu)
                    nc.vector.tensor_tensor(
                        out=ht_sb[:, i * cap + boff:i * cap + boff + BL],
                        in0=sact[:, :BL], in1=psb[:, :BL], op=MUL)
                if i == 0:
                    # ---- A: router logits -> gates (dense per-token) ----
                    for ti, (toff, TL) in enumerate(tiles):
                        psl = psL.tile([128, 512], f32, tag="l", name="psl",
                                       space="PSUM")
                        for k in range(KT):
                            nc.tensor.matmul(
                                psl[:TL, 0:16],
                                lhsT=xgb_sb[:, k * cap + toff:
                                            k * cap + toff + TL],
                                rhs=rwe_sb[:, k * 16:(k + 1) * 16],
                                start=(k == 0), stop=(k == KT - 1))
                        nc.vector.tensor_tensor(
                            out=gates_sb[:TL, ti:ti + 1],
                            in0=psl[:TL, 0:1], in1=msk_sb[:TL, ti:ti + 1],
                            op=MUL)

            # ---- C: y = gate * (ht.T @ w2), scatter rows into acc ----
            for ti, (toff, TL) in enumerate(tiles):
                ysb = work.tile([128, H], bf16, tag="ysb", name="ysb")
                for hh in range(2):
                    psy = psY.tile([128, 512], f32, tag="y", name="psy",
                                   space="PSUM")
                    for i in range(IT):
                        nc.tensor.matmul(
                            psy[:TL, :],
                            lhsT=ht_sb[:, i * cap + toff:i * cap + toff + TL],
                            rhs=w2_sb[:, i * H + hh * 512:
                                      i * H + hh * 512 + 512],
                            start=(i == 0), stop=(i == IT - 1))
                    nc.scalar.activation(
                        ysb[:TL, hh * 512:(hh + 1) * 512],
                        psy[:TL, :], AF.Copy,
                        scale=gates_sb[:TL, ti:ti + 1])
                nc.gpsimd.indirect_dma_start(
                    out=acc[:, :],
                    out_offset=bass.IndirectOffsetOnAxis(
                        ap=idx_sb[:TL, ti:ti + 1], axis=0),
                    in_=ysb[:TL, :], in_offset=None)

            # shared-expert weight loads reuse the expert-weight buffers
            # (single DMAs; arrive during B/C, needed only at S)
            s1_sb = wbig.tile([128, IT * KT * 128], bf16, tag="wa",
                              name="s1_sb")
            nc.sync.dma_start(s1_sb[:], s1i[:, :])
            s3_sb = wbig.tile([128, IT * KT * 128], bf16, tag="wb",
                              name="s3_sb")
            nc.sync.dma_start(s3_sb[:], s3i[:, :])
            s2_sb = wbig.tile([128, IT * H], bf16, tag="wc", name="s2_sb")
            nc.sync.dma_start(s2_sb[:], s2i[:, :])

            # ---- RS: expert-only combine; overlaps with S below ----
            if use_cc:
                nc.gpsimd.collective_compute(
                    "ReduceScatter",
                    mybir.AluOpType.add,
                    replica_groups=[list(range(NCORES))],
                    ins=[acc[:, :]],
                    outs=[rst[:, :]],
                )
                src_t = rst
            else:
                src_t = acc

            # ---- S: shared expert, full I, own 256 tokens ----
            hso = sres.tile([128, IT * TSL], bf16, tag="hso", name="hso")
            for i in range(IT):
                psa = psA.tile([128, 512], f32, tag="a", name="psa_s",
                               space="PSUM")
                psb = psB.tile([128, 512], f32, tag="b", name="psb_s",
                               space="PSUM")
                for k in range(KT):
                    nc.tensor.matmul(
                        psa[:, :TSL],
                        lhsT=s1_sb[:, (i * KT + k) * 128:
                                   (i * KT + k + 1) * 128],
                        rhs=xo_sb[:, k * TSL:(k + 1) * TSL],
                        start=(k == 0), stop=(k == KT - 1))
                for k in range(KT):
                    nc.tensor.matmul(
                        psb[:, :TSL],
                        lhsT=s3_sb[:, (i * KT + k) * 128:
                                   (i * KT + k + 1) * 128],
                        rhs=xo_sb[:, k * TSL:(k + 1) * TSL],
                        start=(k == 0), stop=(k == KT - 1))
                sact = work.tile([128, 512], f32, tag="sact", name="sact_s")
                nc.scalar.activation(sact[:, :TSL], psa[:, :TSL], AF.Silu)
                nc.vector.tensor_tensor(
                    out=hso[:, i * TSL:(i + 1) * TSL],
                    in0=sact[:, :TSL], in1=psb[:, :TSL], op=MUL)
            hfin = sres.tile([128, 2 * H], bf16, tag="hfin", name="hfin")
            for t2 in range(2):
                for hh in range(2):
                    psy = psY.tile([128, 512], f32, tag="y", name="psy_s",
                                   space="PSUM")
                    for i in range(IT):
                        nc.tensor.matmul(
                            psy[:],
                            lhsT=hso[:, i * TSL + t2 * 128:
                                     i * TSL + t2 * 128 + 128],
                            rhs=s2_sb[:, i * H + hh * 512:
                                      i * H + hh * 512 + 512],
                            start=(i == 0), stop=(i == IT - 1))
                    # DVE copy keeps Act on the Silu table (no ATL swap)
                    nc.vector.tensor_copy(
                        hfin[:, t2 * H + hh * 512:t2 * H + (hh + 1) * 512],
                        psy[:])

            # ---- F: out = rst + hfin ----
            rsb = work.tile([128, 2 * H], bf16, tag="rsb", name="rsb",
                            bufs=1)
            nc.sync.dma_start(rsb[:],
                              src_t[0:TSL, :]
                              .rearrange("(c p) h -> p c h", p=128))
            obuf = work.tile([128, 2 * H], bf16, tag="obuf", name="obuf",
                             bufs=1)
            nc.vector.tensor_add(obuf[:], rsb[:], hfin[:])
            nc.sync.dma_start(
                out[:, :].rearrange("(c p) h -> p c h", p=128), obuf[:])

    nc.finalize()
    return nc


def _count_max(x2, router_w):
    logits = x2 @ router_w.T
    order = np.argsort(-logits, axis=1, kind="stable")[:, :TOPK]
    return max(int((order == e).any(axis=1).sum()) for e in range(E))


def _dispatch(x2, router_w, cap=None):
    """Host-side sharding decision: per-expert compact token lists."""
    cap = cap or C
    logits = x2 @ router_w.T                      # [T, E] fp32, dispatch only
    order = np.argsort(-logits, axis=1, kind="stable")[:, :TOPK]
    per_core = []
    all_rows = np.arange(T)
    for e in range(E):
        rows = all_rows[(order == e).any(axis=1)]
        ce = len(rows)
        assert ce <= cap, f"expert {e} overflow: {ce} > {cap}"
        unused = np.setdiff1d(all_rows, rows, assume_unique=True)
        pad = np.resize(unused, cap - ce) if cap > ce else unused[:0]
        idx_full = np.concatenate([rows, pad]).astype(np.int32)
        mask = (np.arange(cap) < ce).astype(np.float32)
        per_core.append((idx_full, mask))
    return per_core


def _make_in_maps(x2, router_w, w1, w2, w3, sw1, sw2, sw3, cap=None):
    import ml_dtypes
    bf16 = ml_dtypes.bfloat16

    cap = cap or C
    tiles, _ = _cap_geom(cap)
    NT = len(tiles)
    dispatch = _dispatch(x2, router_w, cap)

    def upimg(w):
        # [I, H] -> [128, IT*KT*128]; img[p, (i*KT+k)*128+m] = w[i*128+m, k*128+p]
        return np.ascontiguousarray(
            np.asarray(w, np.float32).reshape(IT, 128, KT, 128)
            .transpose(3, 0, 2, 1).reshape(128, IT * KT * 128).astype(bf16))

    def dnimg(w):
        # [H, I] -> [128, IT*H]; img[p, i*H+h] = w[h, i*128+p]
        return np.ascontiguousarray(
            np.asarray(w, np.float32).T.reshape(IT, 128, H)
            .transpose(1, 0, 2).reshape(128, IT * H).astype(bf16))

    x2b = x2.astype(bf16)
    # xo[e][p, k*TSL+t] = x2[e*TSL+t, k*128+p]
    xo_all = np.ascontiguousarray(
        x2b.reshape(E, TSL, KT, 128).transpose(0, 3, 2, 1)
        .reshape(E, 128, KT * TSL))
    s1img = upimg(sw1)
    s3img = upimg(sw3)
    s2img = dnimg(sw2)
    rw = np.asarray(router_w, np.float32)

    in_maps = []
    for e in range(E):
        idx_full, mask = dispatch[e]
        xg = x2b[idx_full]                         # [cap, H] bf16
        xgb_img = np.ascontiguousarray(
            xg.reshape(cap, KT, 128).transpose(2, 1, 0)
            .reshape(128, KT * cap))
        rwe_img = np.ascontiguousarray(
            np.repeat(rw[e].reshape(KT, 128).T[:, :, None], 16, axis=2)
            .reshape(128, KT * 16).astype(bf16))
        ipad = np.zeros(NT * 128, np.int32)
        ipad[:cap] = idx_full
        mpad = np.zeros(NT * 128, np.float32)
        mpad[:cap] = mask
        in_maps.append({
            "xgb": xgb_img,
            "w1i": upimg(w1[e]),
            "w3i": upimg(w3[e]),
            "w2i": dnimg(w2[e]),
            "s1i": s1img,
            "s3i": s3img,
            "s2i": s2img,
            "xo": xo_all[e],
            "rwe": rwe_img,
            "idx": np.ascontiguousarray(ipad.reshape(NT, 128).T),
            "msk": np.ascontiguousarray(mpad.reshape(NT, 128).T),
        })
    return in_maps


def kernel(x, router_w, w1, w2, w3, sw1, sw2, sw3):
    from concourse.bass_utils import run_bass_kernel_spmd

    in_dtype = x.dtype
    x2 = np.ascontiguousarray(x.reshape(T, H), dtype=np.float32)
    router_w = np.asarray(router_w, dtype=np.float32)
    cap = C
    cmax = _count_max(x2, router_w)
    if cmax > C:   # unlikely re-routed inputs: rebuild with a larger capacity
        cap = -((-cmax) // 64) * 64
    key = (1, cap)
    if key not in _BUILD_CACHE:
        _BUILD_CACHE[key] = _build(1, cap=cap)
    nc = _BUILD_CACHE[key]

    in_maps = _make_in_maps(x2, router_w, w1, w2, w3, sw1, sw2, sw3, cap)
    res = run_bass_kernel_spmd(nc, in_maps, list(range(NCORES)))
    out = np.concatenate(
        [np.asarray(res.results[i]["out"], dtype=np.float32)
         for i in range(NCORES)], axis=0)
    return out.reshape(x.shape).astype(in_dtype)


# revision 10
# speedup vs baseline: 1.4802x; 1.0557x over previous
"""MoE (8 experts, top-2, shared expert) Trainium2 kernel.

Expert-parallel over 8 NeuronCores, bf16 matmuls (fp32 PSUM accumulate).
The host performs only the dispatch decision (top-2 expert ids -> compact
per-expert token lists) and data layout (every tensor pre-arranged into its
exact SBUF image so each load is one large contiguous DMA); all model FP
math — router logits, gates, expert SwiGLU, shared expert, cross-core
combine — runs on device.

Device program per core (SPMD, identical program, per-core data):
  A:  router logits for compact tokens (matmul) * validity mask -> gates
  B:  ht[I, C] = silu(w1 @ xg) * (w3 @ xg)          (compact tokens)
  C:  y[ct] = gate * (ht.T @ w2) -> eacc (dense compact order, bf16)
  G:  acc[t] = eacc[inv[t]]  (indirect gather with zero-row sentinel for
      tokens not routed to this core; SBUF bounce, write to acc)
  RS: ReduceScatter(add, bf16) over acc -> rst (this core's 256 rows);
      carries ONLY expert contributions, so it overlaps with...
  S:  shared expert (full I) for this core's OWN 256 tokens -> hfin
  F:  out = rst + hfin   (bf16; host upcasts to f32)

(An indirect SCATTER of eacc rows into a pre-zeroed acc models much worse:
the cost model charges a scatter by the full destination tensor size —
5 x 11.6us — so the gather direction is the cheap one.)
"""

import numpy as np

H = 1024          # hidden
I = 1408          # moe intermediate
E = 8             # experts == cores
T = 2048          # tokens (2*1024)
TOPK = 2
C = 576           # compact per-expert token capacity (max observed 540)
ILOC = I // E     # 176 (unused by the kernel; kept for reference)
TSL = T // E      # 256: output token slice per core
KT = H // 128     # 8 contraction tiles over H
IT = I // 128     # 11 tiles over I
NCORES = 8

_BUILD_CACHE = {}


def _cap_geom(cap):
    """Token tiles (offset, len<=128) and B free-dim chunks (<=512)."""
    assert cap % 64 == 0, cap
    tiles = []
    off = 0
    while off < cap:
        L = min(128, cap - off)
        tiles.append((off, L))
        off += L
    chunks = []
    off = 0
    while off < cap:
        L = min(512, cap - off)
        chunks.append((off, L))
        off += L
    return tiles, chunks


def _build(reps=1, use_cc=True, dtype=None, cap=None):
    import concourse.bacc as bacc
    import concourse.bass as bass
    import concourse.mybir as mybir
    from concourse import tile
    from contextlib import ExitStack

    f32 = mybir.dt.float32
    bf16 = mybir.dt.bfloat16
    i32 = mybir.dt.int32
    AF = mybir.ActivationFunctionType
    MUL = mybir.AluOpType.mult

    cap = cap or C
    tiles, chunks = _cap_geom(cap)
    NT = len(tiles)

    nc = bacc.Bacc("TRN2", target_bir_lowering=False, debug=False,
                   num_devices=NCORES)

    # host-prepared SBUF images, one contiguous DMA each
    xgb = nc.declare_dram_parameter("xgb", [128, KT * cap], bf16,
                                    isOutput=False)
    w1i = nc.declare_dram_parameter("w1i", [128, IT * KT * 128], bf16,
                                    isOutput=False)
    w3i = nc.declare_dram_parameter("w3i", [128, IT * KT * 128], bf16,
                                    isOutput=False)
    w2i = nc.declare_dram_parameter("w2i", [128, IT * H], bf16,
                                    isOutput=False)
    s1i = nc.declare_dram_parameter("s1i", [128, IT * KT * 128], bf16,
                                    isOutput=False)
    s3i = nc.declare_dram_parameter("s3i", [128, IT * KT * 128], bf16,
                                    isOutput=False)
    s2i = nc.declare_dram_parameter("s2i", [128, IT * H], bf16,
                                    isOutput=False)
    xo = nc.declare_dram_parameter("xo", [128, KT * TSL], bf16,
                                   isOutput=False)
    rwe = nc.declare_dram_parameter("rwe", [128, KT * 16], bf16,
                                    isOutput=False)
    invi = nc.declare_dram_parameter("invi", [128, T // 128], i32,
                                     isOutput=False)
    msk = nc.declare_dram_parameter("msk", [128, NT], f32, isOutput=False)
    out = nc.declare_dram_parameter("out", [TSL, H], bf16, isOutput=True)

    acc = nc.dram_tensor("acc", [T, H], bf16)
    eacc = nc.dram_tensor("eacc", [cap + 128, H], bf16)
    rst = nc.dram_tensor("rst", [TSL, H], bf16)
    TT = T // 128

    with tile.TileContext(nc) as tc, ExitStack() as ctx:
        sres = ctx.enter_context(tc.tile_pool(name="sres", bufs=1))
        wbig = ctx.enter_context(tc.tile_pool(name="wbig", bufs=1))
        work = ctx.enter_context(tc.tile_pool(name="work", bufs=2))
        psA = ctx.enter_context(tc.tile_pool(name="psA", bufs=2, space="PSUM"))
        psB = ctx.enter_context(tc.tile_pool(name="psB", bufs=2, space="PSUM"))
        psY = ctx.enter_context(tc.tile_pool(name="psY", bufs=2, space="PSUM"))
        psL = ctx.enter_context(tc.tile_pool(name="psL", bufs=1, space="PSUM"))

        for _rep in range(reps):
            # ---- loads, in consumption order ----
            rwe_sb = sres.tile([128, KT * 16], bf16, tag="rwe_sb",
                               name="rwe_sb")
            nc.sync.dma_start(rwe_sb[:], rwe[:, :])
            invi_sb = sres.tile([128, TT], i32, tag="invi_sb",
                                name="invi_sb")
            nc.sync.dma_start(invi_sb[:], invi[:, :])
            msk_sb = sres.tile([128, NT], f32, tag="msk_sb", name="msk_sb")
            nc.sync.dma_start(msk_sb[:], msk[:, :])
            xgb_sb = sres.tile([128, KT * cap], bf16, tag="xgb_sb",
                               name="xgb_sb")
            half = (KT // 2) * cap
            nc.sync.dma_start(xgb_sb[:, 0:half], xgb[:, 0:half])
            nc.sync.dma_start(xgb_sb[:, half:], xgb[:, half:])
            # expert weights: streamed per-i so B starts after ~0.5 MB
            w1_sb = wbig.tile([128, IT * KT * 128], bf16, tag="wa",
                              name="w1_sb")
            w3_sb = wbig.tile([128, IT * KT * 128], bf16, tag="wb",
                              name="w3_sb")
            for i in range(IT):
                sl = slice(i * KT * 128, (i + 1) * KT * 128)
                nc.sync.dma_start(w1_sb[:, sl], w1i[:, sl])
                nc.sync.dma_start(w3_sb[:, sl], w3i[:, sl])
            # zero sentinel row block for the combine gather
            ztile = work.tile([128, H], bf16, tag="ztile", name="ztile",
                              bufs=1)
            nc.gpsimd.memset(ztile[:], 0.0)
            nc.sync.dma_start(eacc[cap:cap + 128, :], ztile[:])
            w2_sb = wbig.tile([128, IT * H], bf16, tag="wc", name="w2_sb")
            nc.sync.dma_start(w2_sb[:], w2i[:, :])
            xo_sb = sres.tile([128, KT * TSL], bf16, tag="xo_sb",
                              name="xo_sb")
            nc.sync.dma_start(xo_sb[:], xo[:, :])

            gates_sb = sres.tile([128, NT], f32, tag="gates_sb",
                                 name="gates_sb")
            ht_sb = sres.tile([128, IT * cap], bf16, tag="ht_sb",
                              name="ht_sb")

            # ---- B: expert ht = silu(w1@xg)*(w3@xg); A after first i ----
            for i in range(IT):
                for (boff, BL) in chunks:
                    psa = psA.tile([128, 512], f32, tag="a", name="psa",
                                   space="PSUM")
                    psb = psB.tile([128, 512], f32, tag="b", name="psb",
                                   space="PSUM")
                    for k in range(KT):
                        nc.tensor.matmul(
                            psa[:, :BL],
                            lhsT=w1_sb[:, (i * KT + k) * 128:
                                       (i * KT + k + 1) * 128],
                            rhs=xgb_sb[:, k * cap + boff:k * cap + boff + BL],
                            start=(k == 0), stop=(k == KT - 1))
                    for k in range(KT):
                        nc.tensor.matmul(
                            psb[:, :BL],
                            lhsT=w3_sb[:, (i * KT + k) * 128:
                                       (i * KT + k + 1) * 128],
                            rhs=xgb_sb[:, k * cap + boff:k * cap + boff + BL],
                            start=(k == 0), stop=(k == KT - 1))
                    sact = work.tile([128, 512], f32, tag="sact",
                                     name="sact")
                    nc.scalar.activation(sact[:, :BL], psa[:, :BL], AF.Silu)
                    nc.vector.tensor_tensor(
                        out=ht_sb[:, i * cap + boff:i * cap + boff + BL],
                        in0=sact[:, :BL], in1=psb[:, :BL], op=MUL)
                if i == 0:
                    # ---- A: router logits -> gates (dense per-token) ----
                    for ti, (toff, TL) in enumerate(tiles):
                        psl = psL.tile([128, 512], f32, tag="l", name="psl",
                                       space="PSUM")
                        for k in range(KT):
                            nc.tensor.matmul(
                                psl[:TL, 0:16],
                                lhsT=xgb_sb[:, k * cap + toff:
                                            k * cap + toff + TL],
                                rhs=rwe_sb[:, k * 16:(k + 1) * 16],
                                start=(k == 0), stop=(k == KT - 1))
                        nc.vector.tensor_tensor(
                            out=gates_sb[:TL, ti:ti + 1],
                            in0=psl[:TL, 0:1], in1=msk_sb[:TL, ti:ti + 1],
                            op=MUL)

            # shared up-proj weights: single DMAs, WAR on B's last w1/w3
            # reads (SP.SEQ holds while waiting; nothing later needs SP
            # until the gather pass)
            s1_sb = wbig.tile([128, IT * KT * 128], bf16, tag="wa",
                              name="s1_sb")
            nc.sync.dma_start(s1_sb[:], s1i[:, :])
            s3_sb = wbig.tile([128, IT * KT * 128], bf16, tag="wb",
                              name="s3_sb")
            nc.sync.dma_start(s3_sb[:], s3i[:, :])

            # ---- C: y = gate * (ht.T @ w2) -> eacc (compact order) ----
            for ti, (toff, TL) in enumerate(tiles):
                ysb = work.tile([128, H], bf16, tag="ysb", name="ysb")
                for hh in range(2):
                    psy = psY.tile([128, 512], f32, tag="y", name="psy",
                                   space="PSUM")
                    for i in range(IT):
                        nc.tensor.matmul(
                            psy[:TL, :],
                            lhsT=ht_sb[:, i * cap + toff:i * cap + toff + TL],
                            rhs=w2_sb[:, i * H + hh * 512:
                                      i * H + hh * 512 + 512],
                            start=(i == 0), stop=(i == IT - 1))
                    nc.scalar.activation(
                        ysb[:TL, hh * 512:(hh + 1) * 512],
                        psy[:TL, :], AF.Copy,
                        scale=gates_sb[:TL, ti:ti + 1])
                # issued from the Act queue: keeps SP free for weight loads
                nc.scalar.dma_start(eacc[toff:toff + TL, :], ysb[:TL, :])

            s2_sb = wbig.tile([128, IT * H], bf16, tag="wc", name="s2_sb")
            nc.sync.dma_start(s2_sb[:], s2i[:, :])

            # ---- G: acc[t] = eacc[inv[t]]  (gather, 256 rows per DMA) ----
            GR = 1   # token tiles per gather
            for tg in range(TT // GR):
                geacc = work.tile([128, GR * H], bf16, tag="geacc",
                                  name="geacc", bufs=3)
                nc.gpsimd.indirect_dma_start(
                    out=geacc[:], out_offset=None,
                    in_=eacc[:, :],
                    in_offset=bass.IndirectOffsetOnAxis(
                        ap=invi_sb[:, tg * GR:(tg + 1) * GR], axis=0))
                nc.sync.dma_start(
                    acc[tg * GR * 128:(tg + 1) * GR * 128, :]
                    .rearrange("(c p) h -> p c h", p=128),
                    geacc[:])

            # ---- RS: expert-only combine; overlaps with S below ----
            if use_cc:
                nc.gpsimd.collective_compute(
                    "ReduceScatter",
                    mybir.AluOpType.add,
                    replica_groups=[list(range(NCORES))],
                    ins=[acc[:, :]],
                    outs=[rst[:, :]],
                )
                src_t = rst
            else:
                src_t = acc

            # ---- S: shared expert, full I, own 256 tokens ----
            hso = sres.tile([128, IT * TSL], bf16, tag="hso", name="hso")
            for i in range(IT):
                psa = psA.tile([128, 512], f32, tag="a", name="psa_s",
                               space="PSUM")
                psb = psB.tile([128, 512], f32, tag="b", name="psb_s",
                               space="PSUM")
                for k in range(KT):
                    nc.tensor.matmul(
                        psa[:, :TSL],
                        lhsT=s1_sb[:, (i * KT + k) * 128:
                                   (i * KT + k + 1) * 128],
                        rhs=xo_sb[:, k * TSL:(k + 1) * TSL],
                        start=(k == 0), stop=(k == KT - 1))
                for k in range(KT):
                    nc.tensor.matmul(
                        psb[:, :TSL],
                        lhsT=s3_sb[:, (i * KT + k) * 128:
                                   (i * KT + k + 1) * 128],
                        rhs=xo_sb[:, k * TSL:(k + 1) * TSL],
                        start=(k == 0), stop=(k == KT - 1))
                sact = work.tile([128, 512], f32, tag="sact", name="sact_s")
                nc.scalar.activation(sact[:, :TSL], psa[:, :TSL], AF.Silu)
                nc.vector.tensor_tensor(
                    out=hso[:, i * TSL:(i + 1) * TSL],
                    in0=sact[:, :TSL], in1=psb[:, :TSL], op=MUL)
            hfin = sres.tile([128, 2 * H], bf16, tag="hfin", name="hfin")
            for t2 in range(2):
                for hh in range(2):
                    psy = psY.tile([128, 512], f32, tag="y", name="psy_s",
                                   space="PSUM")
                    for i in range(IT):
                        nc.tensor.matmul(
                            psy[:],
                            lhsT=hso[:, i * TSL + t2 * 128:
                                     i * TSL + t2 * 128 + 128],
                            rhs=s2_sb[:, i * H + hh * 512:
                                      i * H + hh * 512 + 512],
                            start=(i == 0), stop=(i == IT - 1))
                    # DVE copy keeps Act on the Silu table (no ATL swap)
                    nc.vector.tensor_copy(
                        hfin[:, t2 * H + hh * 512:t2 * H + (hh + 1) * 512],
                        psy[:])

            # ---- F: out = rst + hfin ----
            rsb = work.tile([128, 2 * H], bf16, tag="rsb", name="rsb",
                            bufs=1)
            nc.sync.dma_start(rsb[:],
                              src_t[0:TSL, :]
                              .rearrange("(c p) h -> p c h", p=128))
            obuf = work.tile([128, 2 * H], bf16, tag="obuf", name="obuf",
                             bufs=1)
            nc.vector.tensor_add(obuf[:], rsb[:], hfin[:])
            nc.sync.dma_start(
                out[:, :].rearrange("(c p) h -> p c h", p=128), obuf[:])

    nc.finalize()
    return nc


def _count_max(x2, router_w):
    logits = x2 @ router_w.T
    order = np.argsort(-logits, axis=1, kind="stable")[:, :TOPK]
    return max(int((order == e).any(axis=1).sum()) for e in range(E))


def _dispatch(x2, router_w, cap=None):
    """Host-side sharding decision: per-expert compact token lists."""
    cap = cap or C
    logits = x2 @ router_w.T                      # [T, E] fp32, dispatch only
    order = np.argsort(-logits, axis=1, kind="stable")[:, :TOPK]
    per_core = []
    all_rows = np.arange(T)
    for e in range(E):
        rows = all_rows[(order == e).any(axis=1)]
        ce = len(rows)
        assert ce <= cap, f"expert {e} overflow: {ce} > {cap}"
        unused = np.setdiff1d(all_rows, rows, assume_unique=True)
        pad = np.resize(unused, cap - ce) if cap > ce else unused[:0]
        idx_full = np.concatenate([rows, pad]).astype(np.int32)
        mask = (np.arange(cap) < ce).astype(np.float32)
        per_core.append((idx_full, mask))
    return per_core


def _make_in_maps(x2, router_w, w1, w2, w3, sw1, sw2, sw3, cap=None):
    import ml_dtypes
    bf16 = ml_dtypes.bfloat16

    cap = cap or C
    tiles, _ = _cap_geom(cap)
    NT = len(tiles)
    dispatch = _dispatch(x2, router_w, cap)

    def upimg(w):
        # [I, H] -> [128, IT*KT*128]; img[p, (i*KT+k)*128+m] = w[i*128+m, k*128+p]
        return np.ascontiguousarray(
            np.asarray(w, np.float32).reshape(IT, 128, KT, 128)
            .transpose(3, 0, 2, 1).reshape(128, IT * KT * 128).astype(bf16))

    def dnimg(w):
        # [H, I] -> [128, IT*H]; img[p, i*H+h] = w[h, i*128+p]
        return np.ascontiguousarray(
            np.asarray(w, np.float32).T.reshape(IT, 128, H)
            .transpose(1, 0, 2).reshape(128, IT * H).astype(bf16))

    x2b = x2.astype(bf16)
    # xo[e][p, k*TSL+t] = x2[e*TSL+t, k*128+p]
    xo_all = np.ascontiguousarray(
        x2b.reshape(E, TSL, KT, 128).transpose(0, 3, 2, 1)
        .reshape(E, 128, KT * TSL))
    s1img = upimg(sw1)
    s3img = upimg(sw3)
    s2img = dnimg(sw2)
    rw = np.asarray(router_w, np.float32)

    in_maps = []
    for e in range(E):
        idx_full, mask = dispatch[e]
        xg = x2b[idx_full]                         # [cap, H] bf16
        xgb_img = np.ascontiguousarray(
            xg.reshape(cap, KT, 128).transpose(2, 1, 0)
            .reshape(128, KT * cap))
        rwe_img = np.ascontiguousarray(
            np.repeat(rw[e].reshape(KT, 128).T[:, :, None], 16, axis=2)
            .reshape(128, KT * 16).astype(bf16))
        mpad = np.zeros(NT * 128, np.float32)
        mpad[:cap] = mask
        ce = int(mask.sum())
        inv = np.full(T, cap, dtype=np.int32)      # sentinel -> zero row
        inv[idx_full[:ce]] = np.arange(ce, dtype=np.int32)
        in_maps.append({
            "xgb": xgb_img,
            "w1i": upimg(w1[e]),
            "w3i": upimg(w3[e]),
            "w2i": dnimg(w2[e]),
            "s1i": s1img,
            "s3i": s3img,
            "s2i": s2img,
            "xo": xo_all[e],
            "rwe": rwe_img,
            "invi": np.ascontiguousarray(inv.reshape(T // 128, 128).T),
            "msk": np.ascontiguousarray(mpad.reshape(NT, 128).T),
        })
    return in_maps


def kernel(x, router_w, w1, w2, w3, sw1, sw2, sw3):
    from concourse.bass_utils import run_bass_kernel_spmd

    in_dtype = x.dtype
    x2 = np.ascontiguousarray(x.reshape(T, H), dtype=np.float32)
    router_w = np.asarray(router_w, dtype=np.float32)
    cap = C
    cmax = _count_max(x2, router_w)
    if cmax > C:   # unlikely re-routed inputs: rebuild with a larger capacity
        cap = -((-cmax) // 64) * 64
    key = (1, cap)
    if key not in _BUILD_CACHE:
        _BUILD_CACHE[key] = _build(1, cap=cap)
    nc = _BUILD_CACHE[key]

    in_maps = _make_in_maps(x2, router_w, w1, w2, w3, sw1, sw2, sw3, cap)
    res = run_bass_kernel_spmd(nc, in_maps, list(range(NCORES)))
    out = np.concatenate(
        [np.asarray(res.results[i]["out"], dtype=np.float32)
         for i in range(NCORES)], axis=0)
    return out.reshape(x.shape).astype(in_dtype)


# revision 14
# speedup vs baseline: 1.7601x; 1.1891x over previous
"""MoE (8 experts, top-2, shared expert) Trainium2 kernel.

Expert-parallel over 8 NeuronCores, bf16 matmuls (fp32 PSUM accumulate).
The host performs only the dispatch decision (top-2 expert ids -> compact
per-expert token lists) and data layout (every tensor pre-arranged into its
exact SBUF image so each load is one large contiguous DMA); all model FP
math — router logits, gates, expert SwiGLU, shared expert, cross-core
combine — runs on device.

Device program per core (SPMD, identical program, per-core data):
  A:  router logits for compact tokens (matmul) * validity mask -> gates
  B:  ht[I, C] = silu(w1 @ xg) * (w3 @ xg)          (compact tokens)
  C:  y[ct] = gate * (ht.T @ w2) -> eacc (dense compact order, bf16)
  G:  acc[t] = eacc[inv[t]]  (indirect gather with zero-row sentinel for
      tokens not routed to this core; SBUF bounce, write to acc)
  RS: ReduceScatter(add, bf16) over acc -> rst (this core's 256 rows);
      carries ONLY expert contributions, so it overlaps with...
  S:  shared expert (full I) for this core's OWN 256 tokens -> hfin
  F:  out = rst + hfin   (bf16; host upcasts to f32)

(An indirect SCATTER of eacc rows into a pre-zeroed acc models much worse:
the cost model charges a scatter by the full destination tensor size —
5 x 11.6us — so the gather direction is the cheap one.)
"""

import numpy as np

H = 1024          # hidden
I = 1408          # moe intermediate
E = 8             # experts == cores
T = 2048          # tokens (2*1024)
TOPK = 2
C = 576           # compact per-expert token capacity (max observed 540)
ILOC = I // E     # 176 (unused by the kernel; kept for reference)
TSL = T // E      # 256: output token slice per core
KT = H // 128     # 8 contraction tiles over H
IT = I // 128     # 11 tiles over I
NCORES = 8

_BUILD_CACHE = {}


def _cap_geom(cap):
    """Token tiles (offset, len<=128) and B free-dim chunks (<=512)."""
    assert cap % 64 == 0, cap
    tiles = []
    off = 0
    while off < cap:
        L = min(128, cap - off)
        tiles.append((off, L))
        off += L
    chunks = []
    off = 0
    while off < cap:
        L = min(512, cap - off)
        chunks.append((off, L))
        off += L
    return tiles, chunks


def _build(reps=1, use_cc=True, dtype=None, cap=None):
    import concourse.bacc as bacc
    import concourse.bass as bass
    import concourse.mybir as mybir
    from concourse import tile
    from contextlib import ExitStack

    f32 = mybir.dt.float32
    bf16 = mybir.dt.bfloat16
    i32 = mybir.dt.int32
    AF = mybir.ActivationFunctionType
    MUL = mybir.AluOpType.mult

    cap = cap or C
    tiles, chunks = _cap_geom(cap)
    NT = len(tiles)

    nc = bacc.Bacc("TRN2", target_bir_lowering=False, debug=False,
                   num_devices=NCORES)

    # host-prepared SBUF images, one contiguous DMA each
    xgb = nc.declare_dram_parameter("xgb", [128, KT * cap], bf16,
                                    isOutput=False)
    w1i = nc.declare_dram_parameter("w1i", [128, IT * KT * 128], bf16,
                                    isOutput=False)
    w3i = nc.declare_dram_parameter("w3i", [128, IT * KT * 128], bf16,
                                    isOutput=False)
    w2i = nc.declare_dram_parameter("w2i", [128, IT * H], bf16,
                                    isOutput=False)
    s1i = nc.declare_dram_parameter("s1i", [128, IT * KT * 128], bf16,
                                    isOutput=False)
    s3i = nc.declare_dram_parameter("s3i", [128, IT * KT * 128], bf16,
                                    isOutput=False)
    s2i = nc.declare_dram_parameter("s2i", [128, IT * H], bf16,
                                    isOutput=False)
    xo = nc.declare_dram_parameter("xo", [128, KT * TSL], bf16,
                                   isOutput=False)
    rwe = nc.declare_dram_parameter("rwe", [128, KT * 16], bf16,
                                    isOutput=False)
    invi = nc.declare_dram_parameter("invi", [128, T // 128], i32,
                                     isOutput=False)
    msk = nc.declare_dram_parameter("msk", [128, NT], f32, isOutput=False)
    out = nc.declare_dram_parameter("out", [TSL, H], bf16, isOutput=True)

    acc = nc.dram_tensor("acc", [T, H], bf16)
    eacc = nc.dram_tensor("eacc", [cap + 128, H], bf16)
    rst = nc.dram_tensor("rst", [TSL, H], bf16)
    TT = T // 128

    with tile.TileContext(nc) as tc, ExitStack() as ctx:
        sres = ctx.enter_context(tc.tile_pool(name="sres", bufs=1))
        wbig = ctx.enter_context(tc.tile_pool(name="wbig", bufs=1))
        work = ctx.enter_context(tc.tile_pool(name="work", bufs=2))
        psA = ctx.enter_context(tc.tile_pool(name="psA", bufs=2, space="PSUM"))
        psB = ctx.enter_context(tc.tile_pool(name="psB", bufs=2, space="PSUM"))
        psY = ctx.enter_context(tc.tile_pool(name="psY", bufs=2, space="PSUM"))
        psL = ctx.enter_context(tc.tile_pool(name="psL", bufs=1, space="PSUM"))

        for _rep in range(reps):
            # ---- loads, in consumption order ----
            rwe_sb = sres.tile([128, KT * 16], bf16, tag="rwe_sb",
                               name="rwe_sb")
            nc.sync.dma_start(rwe_sb[:], rwe[:, :])
            invi_sb = sres.tile([128, TT], i32, tag="invi_sb",
                                name="invi_sb")
            nc.sync.dma_start(invi_sb[:], invi[:, :])
            msk_sb = sres.tile([128, NT], f32, tag="msk_sb", name="msk_sb")
            nc.sync.dma_start(msk_sb[:], msk[:, :])
            xgb_sb = sres.tile([128, KT * cap], bf16, tag="xgb_sb",
                               name="xgb_sb")
            half = (KT // 2) * cap
            nc.sync.dma_start(xgb_sb[:, 0:half], xgb[:, 0:half])
            nc.sync.dma_start(xgb_sb[:, half:], xgb[:, half:])
            # expert weights: streamed per-i so B starts after ~0.5 MB
            w1_sb = wbig.tile([128, IT * KT * 128], bf16, tag="wa",
                              name="w1_sb")
            w3_sb = wbig.tile([128, IT * KT * 128], bf16, tag="wb",
                              name="w3_sb")
            for i in range(IT):
                sl = slice(i * KT * 128, (i + 1) * KT * 128)
                nc.sync.dma_start(w1_sb[:, sl], w1i[:, sl])
                nc.sync.dma_start(w3_sb[:, sl], w3i[:, sl])
            # shared up-proj weights in their own buffers: load during B
            s1_sb = wbig.tile([128, IT * KT * 128], bf16, tag="sa",
                              name="s1_sb")
            nc.sync.dma_start(s1_sb[:], s1i[:, :])
            s3_sb = wbig.tile([128, IT * KT * 128], bf16, tag="sb",
                              name="s3_sb")
            nc.sync.dma_start(s3_sb[:], s3i[:, :])
            # zero sentinel row block for the combine gather
            ztile = work.tile([128, H], bf16, tag="ztile", name="ztile",
                              bufs=1)
            nc.gpsimd.memset(ztile[:], 0.0)
            nc.sync.dma_start(eacc[cap:cap + 128, :], ztile[:])
            w2_sb = wbig.tile([128, IT * H], bf16, tag="wc", name="w2_sb")
            nc.sync.dma_start(w2_sb[:], w2i[:, :])
            xo_sb = sres.tile([128, KT * TSL], bf16, tag="xo_sb",
                              name="xo_sb")
            nc.sync.dma_start(xo_sb[:], xo[:, :])

            gates_sb = sres.tile([128, NT], f32, tag="gates_sb",
                                 name="gates_sb")
            ht_sb = sres.tile([128, IT * cap], bf16, tag="ht_sb",
                              name="ht_sb")

            # ---- B: expert ht = silu(w1@xg)*(w3@xg); A after first i ----
            for i in range(IT):
                for (boff, BL) in chunks:
                    psa = psA.tile([128, 512], f32, tag="a", name="psa",
                                   space="PSUM")
                    psb = psB.tile([128, 512], f32, tag="b", name="psb",
                                   space="PSUM")
                    for k in range(KT):
                        nc.tensor.matmul(
                            psa[:, :BL],
                            lhsT=w1_sb[:, (i * KT + k) * 128:
                                       (i * KT + k + 1) * 128],
                            rhs=xgb_sb[:, k * cap + boff:k * cap + boff + BL],
                            start=(k == 0), stop=(k == KT - 1))
                    for k in range(KT):
                        nc.tensor.matmul(
                            psb[:, :BL],
                            lhsT=w3_sb[:, (i * KT + k) * 128:
                                       (i * KT + k + 1) * 128],
                            rhs=xgb_sb[:, k * cap + boff:k * cap + boff + BL],
                            start=(k == 0), stop=(k == KT - 1))
                    sact = work.tile([128, 512], f32, tag="sact",
                                     name="sact")
                    nc.scalar.activation(sact[:, :BL], psa[:, :BL], AF.Silu)
                    nc.vector.tensor_tensor(
                        out=ht_sb[:, i * cap + boff:i * cap + boff + BL],
                        in0=sact[:, :BL], in1=psb[:, :BL], op=MUL)
                if i == 0:
                    # ---- A: router logits -> gates (dense per-token) ----
                    for ti, (toff, TL) in enumerate(tiles):
                        psl = psL.tile([128, 512], f32, tag="l", name="psl",
                                       space="PSUM")
                        for k in range(KT):
                            nc.tensor.matmul(
                                psl[:TL, 0:16],
                                lhsT=xgb_sb[:, k * cap + toff:
                                            k * cap + toff + TL],
                                rhs=rwe_sb[:, k * 16:(k + 1) * 16],
                                start=(k == 0), stop=(k == KT - 1))
                        nc.vector.tensor_tensor(
                            out=gates_sb[:TL, ti:ti + 1],
                            in0=psl[:TL, 0:1], in1=msk_sb[:TL, ti:ti + 1],
                            op=MUL)

            # ---- C: y = gate * (ht.T @ w2) -> eacc (compact order) ----
            for ti, (toff, TL) in enumerate(tiles):
                ysb = work.tile([128, H], bf16, tag="ysb", name="ysb")
                for hh in range(2):
                    psy = psY.tile([128, 512], f32, tag="y", name="psy",
                                   space="PSUM")
                    for i in range(IT):
                        nc.tensor.matmul(
                            psy[:TL, :],
                            lhsT=ht_sb[:, i * cap + toff:i * cap + toff + TL],
                            rhs=w2_sb[:, i * H + hh * 512:
                                      i * H + hh * 512 + 512],
                            start=(i == 0), stop=(i == IT - 1))
                    nc.scalar.activation(
                        ysb[:TL, hh * 512:(hh + 1) * 512],
                        psy[:TL, :], AF.Copy,
                        scale=gates_sb[:TL, ti:ti + 1])
                nc.sync.dma_start(eacc[toff:toff + TL, :], ysb[:TL, :])

            # s2 reuses w2's buffer; issued from the Act queue so its WAR
            # wait (C's last w2 read) doesn't block SP's gather-pass writes
            s2_sb = wbig.tile([128, IT * H], bf16, tag="wc", name="s2_sb")
            nc.scalar.dma_start(s2_sb[:], s2i[:, :])

            # ---- G: acc[t] = eacc[inv[t]]  (gather with zero sentinel) ----
            GR = 1   # token tiles per gather
            for tg in range(TT // GR):
                geacc = work.tile([128, GR * H], bf16, tag="geacc",
                                  name="geacc", bufs=8)
                nc.gpsimd.indirect_dma_start(
                    out=geacc[:], out_offset=None,
                    in_=eacc[:, :],
                    in_offset=bass.IndirectOffsetOnAxis(
                        ap=invi_sb[:, tg * GR:(tg + 1) * GR], axis=0))
                nc.sync.dma_start(
                    acc[tg * GR * 128:(tg + 1) * GR * 128, :]
                    .rearrange("(c p) h -> p c h", p=128),
                    geacc[:])

            # ---- RS: expert-only combine; overlaps with S below ----
            if use_cc:
                nc.gpsimd.collective_compute(
                    "ReduceScatter",
                    mybir.AluOpType.add,
                    replica_groups=[list(range(NCORES))],
                    ins=[acc[:, :]],
                    outs=[rst[:, :]],
                )
                src_t = rst
            else:
                src_t = acc

            # ---- S: shared expert, full I, own 256 tokens ----
            hso = sres.tile([128, IT * TSL], bf16, tag="hso", name="hso")
            for i in range(IT):
                psa = psA.tile([128, 512], f32, tag="a", name="psa_s",
                               space="PSUM")
                psb = psB.tile([128, 512], f32, tag="b", name="psb_s",
                               space="PSUM")
                for k in range(KT):
                    nc.tensor.matmul(
                        psa[:, :TSL],
                        lhsT=s1_sb[:, (i * KT + k) * 128:
                                   (i * KT + k + 1) * 128],
                        rhs=xo_sb[:, k * TSL:(k + 1) * TSL],
                        start=(k == 0), stop=(k == KT - 1))
                for k in range(KT):
                    nc.tensor.matmul(
                        psb[:, :TSL],
                        lhsT=s3_sb[:, (i * KT + k) * 128:
                                   (i * KT + k + 1) * 128],
                        rhs=xo_sb[:, k * TSL:(k + 1) * TSL],
                        start=(k == 0), stop=(k == KT - 1))
                sact = work.tile([128, 512], f32, tag="sact", name="sact_s")
                nc.scalar.activation(sact[:, :TSL], psa[:, :TSL], AF.Silu)
                nc.vector.tensor_tensor(
                    out=hso[:, i * TSL:(i + 1) * TSL],
                    in0=sact[:, :TSL], in1=psb[:, :TSL], op=MUL)
            hfin = sres.tile([128, 2 * H], bf16, tag="hfin", name="hfin")
            for t2 in range(2):
                for hh in range(2):
                    psy = psY.tile([128, 512], f32, tag="y", name="psy_s",
                                   space="PSUM")
                    for i in range(IT):
                        nc.tensor.matmul(
                            psy[:],
                            lhsT=hso[:, i * TSL + t2 * 128:
                                     i * TSL + t2 * 128 + 128],
                            rhs=s2_sb[:, i * H + hh * 512:
                                      i * H + hh * 512 + 512],
                            start=(i == 0), stop=(i == IT - 1))
                    # DVE copy keeps Act on the Silu table (no ATL swap)
                    nc.vector.tensor_copy(
                        hfin[:, t2 * H + hh * 512:t2 * H + (hh + 1) * 512],
                        psy[:])

            # ---- F: out = rst + hfin (two halves, pipelined) ----
            for c2 in range(2):
                rsb = work.tile([128, H], bf16, tag="rsb", name="rsb")
                nc.sync.dma_start(rsb[:], src_t[c2 * 128:(c2 + 1) * 128, :])
                obuf = work.tile([128, H], bf16, tag="obuf", name="obuf")
                nc.vector.tensor_add(obuf[:], rsb[:],
                                     hfin[:, c2 * H:(c2 + 1) * H])
                nc.sync.dma_start(out[c2 * 128:(c2 + 1) * 128, :], obuf[:])

    nc.finalize()
    return nc


def _count_max(x2, router_w):
    logits = x2 @ router_w.T
    order = np.argsort(-logits, axis=1, kind="stable")[:, :TOPK]
    return max(int((order == e).any(axis=1).sum()) for e in range(E))


def _dispatch(x2, router_w, cap=None):
    """Host-side sharding decision: per-expert compact token lists."""
    cap = cap or C
    logits = x2 @ router_w.T                      # [T, E] fp32, dispatch only
    order = np.argsort(-logits, axis=1, kind="stable")[:, :TOPK]
    per_core = []
    all_rows = np.arange(T)
    for e in range(E):
        rows = all_rows[(order == e).any(axis=1)]
        ce = len(rows)
        assert ce <= cap, f"expert {e} overflow: {ce} > {cap}"
        unused = np.setdiff1d(all_rows, rows, assume_unique=True)
        pad = np.resize(unused, cap - ce) if cap > ce else unused[:0]
        idx_full = np.concatenate([rows, pad]).astype(np.int32)
        mask = (np.arange(cap) < ce).astype(np.float32)
        per_core.append((idx_full, mask))
    return per_core


def _make_in_maps(x2, router_w, w1, w2, w3, sw1, sw2, sw3, cap=None):
    import ml_dtypes
    bf16 = ml_dtypes.bfloat16

    cap = cap or C
    tiles, _ = _cap_geom(cap)
    NT = len(tiles)
    dispatch = _dispatch(x2, router_w, cap)

    def upimg(w):
        # [I, H] -> [128, IT*KT*128]; img[p, (i*KT+k)*128+m] = w[i*128+m, k*128+p]
        return np.ascontiguousarray(
            np.asarray(w, np.float32).reshape(IT, 128, KT, 128)
            .transpose(3, 0, 2, 1).reshape(128, IT * KT * 128).astype(bf16))

    def dnimg(w):
        # [H, I] -> [128, IT*H]; img[p, i*H+h] = w[h, i*128+p]
        return np.ascontiguousarray(
            np.asarray(w, np.float32).T.reshape(IT, 128, H)
            .transpose(1, 0, 2).reshape(128, IT * H).astype(bf16))

    x2b = x2.astype(bf16)
    # xo[e][p, k*TSL+t] = x2[e*TSL+t, k*128+p]
    xo_all = np.ascontiguousarray(
        x2b.reshape(E, TSL, KT, 128).transpose(0, 3, 2, 1)
        .reshape(E, 128, KT * TSL))
    s1img = upimg(sw1)
    s3img = upimg(sw3)
    s2img = dnimg(sw2)
    rw = np.asarray(router_w, np.float32)

    in_maps = []
    for e in range(E):
        idx_full, mask = dispatch[e]
        xg = x2b[idx_full]                         # [cap, H] bf16
        xgb_img = np.ascontiguousarray(
            xg.reshape(cap, KT, 128).transpose(2, 1, 0)
            .reshape(128, KT * cap))
        rwe_img = np.ascontiguousarray(
            np.repeat(rw[e].reshape(KT, 128).T[:, :, None], 16, axis=2)
            .reshape(128, KT * 16).astype(bf16))
        mpad = np.zeros(NT * 128, np.float32)
        mpad[:cap] = mask
        ce = int(mask.sum())
        inv = np.full(T, cap, dtype=np.int32)      # sentinel -> zero row
        inv[idx_full[:ce]] = np.arange(ce, dtype=np.int32)
        in_maps.append({
            "xgb": xgb_img,
            "w1i": upimg(w1[e]),
            "w3i": upimg(w3[e]),
            "w2i": dnimg(w2[e]),
            "s1i": s1img,
            "s3i": s3img,
            "s2i": s2img,
            "xo": xo_all[e],
            "rwe": rwe_img,
            "invi": np.ascontiguousarray(inv.reshape(T // 128, 128).T),
            "msk": np.ascontiguousarray(mpad.reshape(NT, 128).T),
        })
    return in_maps


def kernel(x, router_w, w1, w2, w3, sw1, sw2, sw3):
    from concourse.bass_utils import run_bass_kernel_spmd

    in_dtype = x.dtype
    x2 = np.ascontiguousarray(x.reshape(T, H), dtype=np.float32)
    router_w = np.asarray(router_w, dtype=np.float32)
    cap = C
    cmax = _count_max(x2, router_w)
    if cmax > C:   # unlikely re-routed inputs: rebuild with a larger capacity
        cap = -((-cmax) // 64) * 64
    key = (1, cap)
    if key not in _BUILD_CACHE:
        _BUILD_CACHE[key] = _build(1, cap=cap)
    nc = _BUILD_CACHE[key]

    in_maps = _make_in_maps(x2, router_w, w1, w2, w3, sw1, sw2, sw3, cap)
    res = run_bass_kernel_spmd(nc, in_maps, list(range(NCORES)))
    out = np.concatenate(
        [np.asarray(res.results[i]["out"], dtype=np.float32)
         for i in range(NCORES)], axis=0)
    return out.reshape(x.shape).astype(in_dtype)


# revision 22
# speedup vs baseline: 2.0278x; 1.1521x over previous
"""MoE (8 experts, top-2, shared expert) Trainium2 kernel.

Expert-parallel over 8 NeuronCores, bf16 matmuls (fp32 PSUM accumulate).
The host performs only the dispatch decision (top-2 expert ids -> compact
per-expert token lists) and data layout (every tensor pre-arranged into its
exact SBUF image so each load is one large contiguous DMA); all model FP
math — router logits, gates, expert SwiGLU, shared expert, cross-core
combine — runs on device.

Device program per core (SPMD, identical program, per-core data):
  A:  router logits for compact tokens (matmul) * validity mask -> gates
  B:  ht[I, C] = silu(w1 @ xg) * (w3 @ xg)          (compact tokens)
  C:  y[ct] = gate * (ht.T @ w2) -> eacc (dense compact order, bf16)
  G:  acc[t] = eacc[inv[t]]  (indirect gather with zero-row sentinel for
      tokens not routed to this core; SBUF bounce, write to acc)
  RS: ReduceScatter(add, bf16) over acc -> rst (this core's 256 rows);
      carries ONLY expert contributions, so it overlaps with...
  S:  shared expert (full I) for this core's OWN 256 tokens -> hfin
  F:  out = rst + hfin   (bf16; host upcasts to f32)

(An indirect SCATTER of eacc rows into a pre-zeroed acc models much worse:
the cost model charges a scatter by the full destination tensor size —
5 x 11.6us — so the gather direction is the cheap one.)
"""

import numpy as np

H = 1024          # hidden
I = 1408          # moe intermediate
E = 8             # experts == cores
T = 2048          # tokens (2*1024)
TOPK = 2
C = 576           # compact per-expert token capacity (max observed 540)
ILOC = I // E     # 176 (unused by the kernel; kept for reference)
TSL = T // E      # 256: output token slice per core
KT = H // 128     # 8 contraction tiles over H
IT = I // 128     # 11 tiles over I
NCORES = 8

_BUILD_CACHE = {}


def _cap_geom(cap):
    """Token tiles (offset, len<=128) and B free-dim chunks (<=512)."""
    assert cap % 64 == 0, cap
    tiles = []
    off = 0
    while off < cap:
        L = min(128, cap - off)
        tiles.append((off, L))
        off += L
    chunks = []
    off = 0
    while off < cap:
        L = min(512, cap - off)
        chunks.append((off, L))
        off += L
    return tiles, chunks


def _build(reps=1, use_cc=True, dtype=None, cap=None, prefixes=None):
    import concourse.bacc as bacc
    import concourse.bass as bass
    import concourse.mybir as mybir
    from concourse import tile
    from contextlib import ExitStack

    f32 = mybir.dt.float32
    bf16 = mybir.dt.bfloat16
    i32 = mybir.dt.int32
    AF = mybir.ActivationFunctionType
    MUL = mybir.AluOpType.mult

    cap = cap or C
    tiles, chunks = _cap_geom(cap)
    NT = len(tiles)

    nc = bacc.Bacc("TRN2", target_bir_lowering=False, debug=False,
                   num_devices=NCORES)

    # host-prepared SBUF images, one contiguous DMA each
    xgb = nc.declare_dram_parameter("xgb", [128, KT * cap], bf16,
                                    isOutput=False)
    w1i = nc.declare_dram_parameter("w1i", [128, IT * KT * 128], bf16,
                                    isOutput=False)
    w3i = nc.declare_dram_parameter("w3i", [128, IT * KT * 128], bf16,
                                    isOutput=False)
    w2i = nc.declare_dram_parameter("w2i", [128, IT * H], bf16,
                                    isOutput=False)
    s1i = nc.declare_dram_parameter("s1i", [128, IT * KT * 128], bf16,
                                    isOutput=False)
    s3i = nc.declare_dram_parameter("s3i", [128, IT * KT * 128], bf16,
                                    isOutput=False)
    s2i = nc.declare_dram_parameter("s2i", [128, IT * H], bf16,
                                    isOutput=False)
    xo = nc.declare_dram_parameter("xo", [128, KT * TSL], bf16,
                                   isOutput=False)
    rwe = nc.declare_dram_parameter("rwe", [128, KT * 16], bf16,
                                    isOutput=False)
    invi = nc.declare_dram_parameter("invi", [128, T // 128], i32,
                                     isOutput=False)
    msk = nc.declare_dram_parameter("msk", [128, NT], f32, isOutput=False)
    out = nc.declare_dram_parameter("out", [TSL, H], bf16, isOutput=True)

    acc = nc.dram_tensor("acc", [T, H], bf16)
    # eacc row 0..127: zero sentinel; compact row c lives at 128+c
    eacc = nc.dram_tensor("eacc", [cap + 128, H], bf16)
    rst = nc.dram_tensor("rst", [TSL, H], bf16)
    TT = T // 128
    if prefixes is None:
        prefixes = (cap + 128,) * TT

    with tile.TileContext(nc) as tc, ExitStack() as ctx:
        sres = ctx.enter_context(tc.tile_pool(name="sres", bufs=1))
        wbig = ctx.enter_context(tc.tile_pool(name="wbig", bufs=1))
        work = ctx.enter_context(tc.tile_pool(name="work", bufs=2))
        psA = ctx.enter_context(tc.tile_pool(name="psA", bufs=2, space="PSUM"))
        psB = ctx.enter_context(tc.tile_pool(name="psB", bufs=2, space="PSUM"))
        psY = ctx.enter_context(tc.tile_pool(name="psY", bufs=2, space="PSUM"))
        psL = ctx.enter_context(tc.tile_pool(name="psL", bufs=1, space="PSUM"))

        for _rep in range(reps):
            # ---- loads, in consumption order (B-critical ones first) ----
            xgb_sb = sres.tile([128, KT * cap], bf16, tag="xgb_sb",
                               name="xgb_sb")
            half = (KT // 2) * cap
            nc.sync.dma_start(xgb_sb[:, 0:half], xgb[:, 0:half])
            # expert weights: streamed per-i so B starts after ~0.5 MB
            w1_sb = wbig.tile([128, IT * KT * 128], bf16, tag="wa",
                              name="w1_sb")
            w3_sb = wbig.tile([128, IT * KT * 128], bf16, tag="wb",
                              name="w3_sb")
            sl = slice(0, KT * 128)
            nc.sync.dma_start(w1_sb[:, sl], w1i[:, sl])
            nc.sync.dma_start(w3_sb[:, sl], w3i[:, sl])
            nc.sync.dma_start(xgb_sb[:, half:], xgb[:, half:])
            rwe_sb = sres.tile([128, KT * 16], bf16, tag="rwe_sb",
                               name="rwe_sb")
            nc.sync.dma_start(rwe_sb[:], rwe[:, :])
            msk_sb = sres.tile([128, NT], f32, tag="msk_sb", name="msk_sb")
            nc.sync.dma_start(msk_sb[:], msk[:, :])
            invi_sb = sres.tile([128, TT], i32, tag="invi_sb",
                                name="invi_sb")
            nc.sync.dma_start(invi_sb[:], invi[:, :])
            for i in range(1, IT):
                sl = slice(i * KT * 128, (i + 1) * KT * 128)
                nc.sync.dma_start(w1_sb[:, sl], w1i[:, sl])
                nc.sync.dma_start(w3_sb[:, sl], w3i[:, sl])
            # zero sentinel rows 0..127 for the combine gather
            ztile = work.tile([128, H], bf16, tag="ztile", name="ztile",
                              bufs=1)
            nc.gpsimd.memset(ztile[:], 0.0)
            nc.sync.dma_start(eacc[0:128, :], ztile[:])
            w2_sb = wbig.tile([128, IT * H], bf16, tag="wc", name="w2_sb")
            nc.sync.dma_start(w2_sb[:], w2i[:, :])
            # shared weights in their own buffers: load during B
            s1_sb = wbig.tile([128, IT * KT * 128], bf16, tag="sa",
                              name="s1_sb")
            nc.sync.dma_start(s1_sb[:], s1i[:, :])
            s3_sb = wbig.tile([128, IT * KT * 128], bf16, tag="sb",
                              name="s3_sb")
            nc.sync.dma_start(s3_sb[:], s3i[:, :])
            s2_sb = wbig.tile([128, IT * H], bf16, tag="sc", name="s2_sb")
            nc.sync.dma_start(s2_sb[:], s2i[:, :])
            xo_sb = sres.tile([128, KT * TSL], bf16, tag="xo_sb",
                              name="xo_sb")
            nc.sync.dma_start(xo_sb[:], xo[:, :])

            gates_sb = sres.tile([128, NT], f32, tag="gates_sb",
                                 name="gates_sb")
            ht_sb = sres.tile([128, IT * cap], bf16, tag="ht_sb",
                              name="ht_sb")

            # ---- B: expert ht = silu(w1@xg)*(w3@xg); A after first i ----
            for i in range(IT):
                for (boff, BL) in chunks:
                    psa = psA.tile([128, 512], f32, tag="a", name="psa",
                                   space="PSUM")
                    psb = psB.tile([128, 512], f32, tag="b", name="psb",
                                   space="PSUM")
                    for k in range(KT):
                        nc.tensor.matmul(
                            psa[:, :BL],
                            lhsT=w1_sb[:, (i * KT + k) * 128:
                                       (i * KT + k + 1) * 128],
                            rhs=xgb_sb[:, k * cap + boff:k * cap + boff + BL],
                            start=(k == 0), stop=(k == KT - 1))
                    for k in range(KT):
                        nc.tensor.matmul(
                            psb[:, :BL],
                            lhsT=w3_sb[:, (i * KT + k) * 128:
                                       (i * KT + k + 1) * 128],
                            rhs=xgb_sb[:, k * cap + boff:k * cap + boff + BL],
                            start=(k == 0), stop=(k == KT - 1))
                    sact = work.tile([128, 512], f32, tag="sact",
                                     name="sact")
                    nc.scalar.activation(sact[:, :BL], psa[:, :BL], AF.Silu)
                    nc.vector.tensor_tensor(
                        out=ht_sb[:, i * cap + boff:i * cap + boff + BL],
                        in0=sact[:, :BL], in1=psb[:, :BL], op=MUL)
                if i == 0:
                    # ---- A: router logits -> gates (dense per-token) ----
                    for ti, (toff, TL) in enumerate(tiles):
                        psl = psL.tile([128, 512], f32, tag="l", name="psl",
                                       space="PSUM")
                        for k in range(KT):
                            nc.tensor.matmul(
                                psl[:TL, 0:16],
                                lhsT=xgb_sb[:, k * cap + toff:
                                            k * cap + toff + TL],
                                rhs=rwe_sb[:, k * 16:(k + 1) * 16],
                                start=(k == 0), stop=(k == KT - 1))
                        nc.vector.tensor_tensor(
                            out=gates_sb[:TL, ti:ti + 1],
                            in0=psl[:TL, 0:1], in1=msk_sb[:TL, ti:ti + 1],
                            op=MUL)

            # ---- C: y = gate * (ht.T @ w2) -> eacc (compact order) ----
            for ti, (toff, TL) in enumerate(tiles):
                ysb = work.tile([128, H], bf16, tag="ysb", name="ysb")
                for hh in range(2):
                    psy = psY.tile([128, 512], f32, tag="y", name="psy",
                                   space="PSUM")
                    for i in range(IT):
                        nc.tensor.matmul(
                            psy[:TL, :],
                            lhsT=ht_sb[:, i * cap + toff:i * cap + toff + TL],
                            rhs=w2_sb[:, i * H + hh * 512:
                                      i * H + hh * 512 + 512],
                            start=(i == 0), stop=(i == IT - 1))
                    nc.scalar.activation(
                        ysb[:TL, hh * 512:(hh + 1) * 512],
                        psy[:TL, :], AF.Copy,
                        scale=gates_sb[:TL, ti:ti + 1])
                nc.scalar.dma_start(eacc[128 + toff:128 + toff + TL, :],
                                    ysb[:TL, :])

            # ---- G: acc[t] = eacc[inv[t]]  (gather with zero sentinel) ----
            # Each gather's source AP is a prefix slice of eacc covering only
            # the rows its tokens can reference (compact order is sorted by
            # token id), so gathers pipeline WITH C instead of after it.
            for tg in range(TT):
                geacc = work.tile([128, H], bf16, tag="geacc",
                                  name="geacc", bufs=6)
                nc.gpsimd.indirect_dma_start(
                    out=geacc[:], out_offset=None,
                    in_=eacc[0:prefixes[tg], :],
                    in_offset=bass.IndirectOffsetOnAxis(
                        ap=invi_sb[:, tg:tg + 1], axis=0))
                nc.sync.dma_start(
                    acc[tg * 128:(tg + 1) * 128, :], geacc[:])

            # ---- RS: expert-only combine; overlaps with S below ----
            if use_cc:
                nc.gpsimd.collective_compute(
                    "ReduceScatter",
                    mybir.AluOpType.add,
                    replica_groups=[list(range(NCORES))],
                    ins=[acc[:, :]],
                    outs=[rst[:, :]],
                )
                src_t = rst
            else:
                src_t = acc

            # ---- S: shared expert, full I, own 256 tokens ----
            hso = sres.tile([128, IT * TSL], bf16, tag="hso", name="hso")
            for i in range(IT):
                psa = psA.tile([128, 512], f32, tag="a", name="psa_s",
                               space="PSUM")
                psb = psB.tile([128, 512], f32, tag="b", name="psb_s",
                               space="PSUM")
                for k in range(KT):
                    nc.tensor.matmul(
                        psa[:, :TSL],
                        lhsT=s1_sb[:, (i * KT + k) * 128:
                                   (i * KT + k + 1) * 128],
                        rhs=xo_sb[:, k * TSL:(k + 1) * TSL],
                        start=(k == 0), stop=(k == KT - 1))
                for k in range(KT):
                    nc.tensor.matmul(
                        psb[:, :TSL],
                        lhsT=s3_sb[:, (i * KT + k) * 128:
                                   (i * KT + k + 1) * 128],
                        rhs=xo_sb[:, k * TSL:(k + 1) * TSL],
                        start=(k == 0), stop=(k == KT - 1))
                sact = work.tile([128, 512], f32, tag="sact", name="sact_s")
                nc.scalar.activation(sact[:, :TSL], psa[:, :TSL], AF.Silu)
                nc.vector.tensor_tensor(
                    out=hso[:, i * TSL:(i + 1) * TSL],
                    in0=sact[:, :TSL], in1=psb[:, :TSL], op=MUL)
            hfin = sres.tile([128, 2 * H], bf16, tag="hfin", name="hfin")
            for t2 in range(2):
                for hh in range(2):
                    psy = psY.tile([128, 512], f32, tag="y", name="psy_s",
                                   space="PSUM")
                    for i in range(IT):
                        nc.tensor.matmul(
                            psy[:],
                            lhsT=hso[:, i * TSL + t2 * 128:
                                     i * TSL + t2 * 128 + 128],
                            rhs=s2_sb[:, i * H + hh * 512:
                                      i * H + hh * 512 + 512],
                            start=(i == 0), stop=(i == IT - 1))
                    # DVE copy keeps Act on the Silu table (no ATL swap)
                    nc.vector.tensor_copy(
                        hfin[:, t2 * H + hh * 512:t2 * H + (hh + 1) * 512],
                        psy[:])

            # ---- F: out = rst + hfin (two halves, pipelined) ----
            for c2 in range(2):
                rsb = work.tile([128, H], bf16, tag="rsb", name="rsb")
                nc.sync.dma_start(rsb[:], src_t[c2 * 128:(c2 + 1) * 128, :])
                obuf = work.tile([128, H], bf16, tag="obuf", name="obuf")
                nc.vector.tensor_add(obuf[:], rsb[:],
                                     hfin[:, c2 * H:(c2 + 1) * H])
                nc.sync.dma_start(out[c2 * 128:(c2 + 1) * 128, :], obuf[:])

    nc.finalize()
    return nc


def _count_max(x2, router_w):
    logits = x2 @ router_w.T
    order = np.argsort(-logits, axis=1, kind="stable")[:, :TOPK]
    return max(int((order == e).any(axis=1).sum()) for e in range(E))


def _dispatch(x2, router_w, cap=None):
    """Host-side sharding decision: per-expert compact token lists."""
    cap = cap or C
    logits = x2 @ router_w.T                      # [T, E] fp32, dispatch only
    order = np.argsort(-logits, axis=1, kind="stable")[:, :TOPK]
    per_core = []
    all_rows = np.arange(T)
    for e in range(E):
        rows = all_rows[(order == e).any(axis=1)]
        ce = len(rows)
        assert ce <= cap, f"expert {e} overflow: {ce} > {cap}"
        unused = np.setdiff1d(all_rows, rows, assume_unique=True)
        pad = np.resize(unused, cap - ce) if cap > ce else unused[:0]
        idx_full = np.concatenate([rows, pad]).astype(np.int32)
        mask = (np.arange(cap) < ce).astype(np.float32)
        per_core.append((idx_full, mask))
    return per_core


def _make_in_maps(x2, router_w, w1, w2, w3, sw1, sw2, sw3, cap=None):
    import ml_dtypes
    bf16 = ml_dtypes.bfloat16

    cap = cap or C
    tiles, _ = _cap_geom(cap)
    NT = len(tiles)
    dispatch = _dispatch(x2, router_w, cap)

    def upimg(w):
        # [I, H] -> [128, IT*KT*128]; img[p, (i*KT+k)*128+m] = w[i*128+m, k*128+p]
        return np.ascontiguousarray(
            np.asarray(w, np.float32).reshape(IT, 128, KT, 128)
            .transpose(3, 0, 2, 1).reshape(128, IT * KT * 128).astype(bf16))

    def dnimg(w):
        # [H, I] -> [128, IT*H]; img[p, i*H+h] = w[h, i*128+p]
        return np.ascontiguousarray(
            np.asarray(w, np.float32).T.reshape(IT, 128, H)
            .transpose(1, 0, 2).reshape(128, IT * H).astype(bf16))

    x2b = x2.astype(bf16)
    # xo[e][p, k*TSL+t] = x2[e*TSL+t, k*128+p]
    xo_all = np.ascontiguousarray(
        x2b.reshape(E, TSL, KT, 128).transpose(0, 3, 2, 1)
        .reshape(E, 128, KT * TSL))
    s1img = upimg(sw1)
    s3img = upimg(sw3)
    s2img = dnimg(sw2)
    rw = np.asarray(router_w, np.float32)

    in_maps = []
    for e in range(E):
        idx_full, mask = dispatch[e]
        xg = x2b[idx_full]                         # [cap, H] bf16
        xgb_img = np.ascontiguousarray(
            xg.reshape(cap, KT, 128).transpose(2, 1, 0)
            .reshape(128, KT * cap))
        rwe_img = np.ascontiguousarray(
            np.repeat(rw[e].reshape(KT, 128).T[:, :, None], 16, axis=2)
            .reshape(128, KT * 16).astype(bf16))
        mpad = np.zeros(NT * 128, np.float32)
        mpad[:cap] = mask
        ce = int(mask.sum())
        inv = np.zeros(T, dtype=np.int32)          # sentinel -> zero row 0
        inv[idx_full[:ce]] = 128 + np.arange(ce, dtype=np.int32)
        in_maps.append({
            "xgb": xgb_img,
            "w1i": upimg(w1[e]),
            "w3i": upimg(w3[e]),
            "w2i": dnimg(w2[e]),
            "s1i": s1img,
            "s3i": s3img,
            "s2i": s2img,
            "xo": xo_all[e],
            "rwe": rwe_img,
            "invi": np.ascontiguousarray(inv.reshape(T // 128, 128).T),
            "msk": np.ascontiguousarray(mpad.reshape(NT, 128).T),
        })
    return in_maps


def _prefixes(x2, router_w, cap):
    """Per token-tile eacc prefix (max over cores, for the shared SPMD
    program): gather tg only reads eacc rows < P[tg]."""
    logits = x2 @ router_w.T
    order = np.argsort(-logits, axis=1, kind="stable")[:, :TOPK]
    TT = T // 128
    P = np.full(TT, 128, dtype=np.int64)
    for e in range(E):
        rows = np.arange(T)[(order == e).any(axis=1)]
        cum = np.searchsorted(rows, (np.arange(TT) + 1) * 128)
        P = np.maximum(P, 128 + cum)
    return tuple(int(min(p, cap + 128)) for p in P)


def _nc_for(x2, router_w, cap=None):
    """The exact module kernel() will run for these inputs (cached)."""
    cap = cap or C
    cmax = _count_max(x2, router_w)
    if cmax > cap:  # unlikely re-routed inputs: rebuild with larger capacity
        cap = -((-cmax) // 64) * 64
    prefixes = _prefixes(x2, router_w, cap)
    key = (1, cap, prefixes)
    if key not in _BUILD_CACHE:
        _BUILD_CACHE[key] = _build(1, cap=cap, prefixes=prefixes)
    return _BUILD_CACHE[key], cap


def kernel(x, router_w, w1, w2, w3, sw1, sw2, sw3):
    from concourse.bass_utils import run_bass_kernel_spmd

    in_dtype = x.dtype
    x2 = np.ascontiguousarray(x.reshape(T, H), dtype=np.float32)
    router_w = np.asarray(router_w, dtype=np.float32)
    nc, cap = _nc_for(x2, router_w)

    in_maps = _make_in_maps(x2, router_w, w1, w2, w3, sw1, sw2, sw3, cap)
    res = run_bass_kernel_spmd(nc, in_maps, list(range(NCORES)))
    out = np.concatenate(
        [np.asarray(res.results[i]["out"], dtype=np.float32)
         for i in range(NCORES)], axis=0)
    return out.reshape(x.shape).astype(in_dtype)


# revision 23
# speedup vs baseline: 2.0381x; 1.0051x over previous
"""MoE (8 experts, top-2, shared expert) Trainium2 kernel.

Expert-parallel over 8 NeuronCores, bf16 matmuls (fp32 PSUM accumulate).
The host performs only the dispatch decision (top-2 expert ids -> compact
per-expert token lists) and data layout (every tensor pre-arranged into its
exact SBUF image so each load is one large contiguous DMA); all model FP
math — router logits, gates, expert SwiGLU, shared expert, cross-core
combine — runs on device.

Device program per core (SPMD, identical program, per-core data):
  A:  router logits for compact tokens (matmul) * validity mask -> gates
  B:  ht[I, C] = silu(w1 @ xg) * (w3 @ xg)          (compact tokens)
  C:  y[ct] = gate * (ht.T @ w2) -> eacc (dense compact order, bf16)
  G:  acc[t] = eacc[inv[t]]  (indirect gather with zero-row sentinel for
      tokens not routed to this core; SBUF bounce, write to acc)
  RS: ReduceScatter(add, bf16) over acc -> rst (this core's 256 rows);
      carries ONLY expert contributions, so it overlaps with...
  S:  shared expert (full I) for this core's OWN 256 tokens -> hfin
  F:  out = rst + hfin   (bf16; host upcasts to f32)

(An indirect SCATTER of eacc rows into a pre-zeroed acc models much worse:
the cost model charges a scatter by the full destination tensor size —
5 x 11.6us — so the gather direction is the cheap one.)
"""

import numpy as np

H = 1024          # hidden
I = 1408          # moe intermediate
E = 8             # experts == cores
T = 2048          # tokens (2*1024)
TOPK = 2
C = 576           # compact per-expert token capacity (max observed 540)
ILOC = I // E     # 176 (unused by the kernel; kept for reference)
TSL = T // E      # 256: output token slice per core
KT = H // 128     # 8 contraction tiles over H
IT = I // 128     # 11 tiles over I
NCORES = 8

_BUILD_CACHE = {}


def _cap_geom(cap):
    """Token tiles (offset, len<=128) and B free-dim chunks (<=512)."""
    assert cap % 64 == 0, cap
    tiles = []
    off = 0
    while off < cap:
        L = min(128, cap - off)
        tiles.append((off, L))
        off += L
    chunks = []
    off = 0
    while off < cap:
        L = min(512, cap - off)
        chunks.append((off, L))
        off += L
    return tiles, chunks


def _build(reps=1, use_cc=True, dtype=None, cap=None, prefixes=None):
    import concourse.bacc as bacc
    import concourse.bass as bass
    import concourse.mybir as mybir
    from concourse import tile
    from contextlib import ExitStack

    f32 = mybir.dt.float32
    bf16 = mybir.dt.bfloat16
    i32 = mybir.dt.int32
    AF = mybir.ActivationFunctionType
    MUL = mybir.AluOpType.mult

    cap = cap or C
    tiles, chunks = _cap_geom(cap)
    NT = len(tiles)

    nc = bacc.Bacc("TRN2", target_bir_lowering=False, debug=False,
                   num_devices=NCORES)

    # host-prepared SBUF images, one contiguous DMA each
    xgb = nc.declare_dram_parameter("xgb", [128, KT * cap], bf16,
                                    isOutput=False)
    w1i = nc.declare_dram_parameter("w1i", [128, IT * KT * 128], bf16,
                                    isOutput=False)
    w3i = nc.declare_dram_parameter("w3i", [128, IT * KT * 128], bf16,
                                    isOutput=False)
    w2i = nc.declare_dram_parameter("w2i", [128, IT * H], bf16,
                                    isOutput=False)
    s1i = nc.declare_dram_parameter("s1i", [128, IT * KT * 128], bf16,
                                    isOutput=False)
    s3i = nc.declare_dram_parameter("s3i", [128, IT * KT * 128], bf16,
                                    isOutput=False)
    s2i = nc.declare_dram_parameter("s2i", [128, IT * H], bf16,
                                    isOutput=False)
    xo = nc.declare_dram_parameter("xo", [128, KT * TSL], bf16,
                                   isOutput=False)
    rwe = nc.declare_dram_parameter("rwe", [128, KT * 16], bf16,
                                    isOutput=False)
    invi = nc.declare_dram_parameter("invi", [128, T // 128], i32,
                                     isOutput=False)
    msk = nc.declare_dram_parameter("msk", [128, NT], f32, isOutput=False)
    out = nc.declare_dram_parameter("out", [TSL, H], bf16, isOutput=True)

    acc = nc.dram_tensor("acc", [T, H], bf16)
    # eacc row 0..127: zero sentinel; compact row c lives at 128+c
    eacc = nc.dram_tensor("eacc", [cap + 128, H], bf16)
    rst = nc.dram_tensor("rst", [TSL, H], bf16)
    TT = T // 128
    if prefixes is None:
        prefixes = (cap + 128,) * TT

    with tile.TileContext(nc) as tc, ExitStack() as ctx:
        sres = ctx.enter_context(tc.tile_pool(name="sres", bufs=1))
        wbig = ctx.enter_context(tc.tile_pool(name="wbig", bufs=1))
        work = ctx.enter_context(tc.tile_pool(name="work", bufs=2))
        psA = ctx.enter_context(tc.tile_pool(name="psA", bufs=2, space="PSUM"))
        psB = ctx.enter_context(tc.tile_pool(name="psB", bufs=2, space="PSUM"))
        psY = ctx.enter_context(tc.tile_pool(name="psY", bufs=2, space="PSUM"))
        psL = ctx.enter_context(tc.tile_pool(name="psL", bufs=1, space="PSUM"))

        for _rep in range(reps):
            # ---- loads, in consumption order (B-critical ones first) ----
            xgb_sb = sres.tile([128, KT * cap], bf16, tag="xgb_sb",
                               name="xgb_sb")
            half = (KT // 2) * cap
            nc.sync.dma_start(xgb_sb[:, 0:half], xgb[:, 0:half])
            # expert weights: streamed per-i so B starts after ~0.5 MB
            w1_sb = wbig.tile([128, IT * KT * 128], bf16, tag="wa",
                              name="w1_sb")
            w3_sb = wbig.tile([128, IT * KT * 128], bf16, tag="wb",
                              name="w3_sb")
            sl = slice(0, KT * 128)
            nc.sync.dma_start(w1_sb[:, sl], w1i[:, sl])
            nc.sync.dma_start(w3_sb[:, sl], w3i[:, sl])
            nc.sync.dma_start(xgb_sb[:, half:], xgb[:, half:])
            rwe_sb = sres.tile([128, KT * 16], bf16, tag="rwe_sb",
                               name="rwe_sb")
            nc.sync.dma_start(rwe_sb[:], rwe[:, :])
            msk_sb = sres.tile([128, NT], f32, tag="msk_sb", name="msk_sb")
            nc.sync.dma_start(msk_sb[:], msk[:, :])
            invi_sb = sres.tile([128, TT], i32, tag="invi_sb",
                                name="invi_sb")
            nc.sync.dma_start(invi_sb[:], invi[:, :])
            for i in range(1, IT):
                sl = slice(i * KT * 128, (i + 1) * KT * 128)
                nc.sync.dma_start(w1_sb[:, sl], w1i[:, sl])
                nc.sync.dma_start(w3_sb[:, sl], w3i[:, sl])
            # zero sentinel rows 0..127 for the combine gather
            ztile = work.tile([128, H], bf16, tag="ztile", name="ztile",
                              bufs=1)
            nc.gpsimd.memset(ztile[:], 0.0)
            nc.sync.dma_start(eacc[0:128, :], ztile[:])
            w2_sb = wbig.tile([128, IT * H], bf16, tag="wc", name="w2_sb")
            nc.sync.dma_start(w2_sb[:], w2i[:, :])
            # shared weights in their own buffers: load during B
            s1_sb = wbig.tile([128, IT * KT * 128], bf16, tag="sa",
                              name="s1_sb")
            nc.sync.dma_start(s1_sb[:], s1i[:, :])
            s3_sb = wbig.tile([128, IT * KT * 128], bf16, tag="sb",
                              name="s3_sb")
            nc.sync.dma_start(s3_sb[:], s3i[:, :])
            s2_sb = wbig.tile([128, IT * H], bf16, tag="sc", name="s2_sb")
            nc.sync.dma_start(s2_sb[:], s2i[:, :])
            xo_sb = sres.tile([128, KT * TSL], bf16, tag="xo_sb",
                              name="xo_sb")
            nc.sync.dma_start(xo_sb[:], xo[:, :])

            gates_sb = sres.tile([128, NT], f32, tag="gates_sb",
                                 name="gates_sb")
            ht_sb = sres.tile([128, IT * cap], bf16, tag="ht_sb",
                              name="ht_sb")

            # ---- B: expert ht = silu(w1@xg)*(w3@xg); A after first i ----
            for i in range(IT):
                for (boff, BL) in chunks:
                    psa = psA.tile([128, 512], f32, tag="a", name="psa",
                                   space="PSUM")
                    psb = psB.tile([128, 512], f32, tag="b", name="psb",
                                   space="PSUM")
                    for k in range(KT):
                        nc.tensor.matmul(
                            psa[:, :BL],
                            lhsT=w1_sb[:, (i * KT + k) * 128:
                                       (i * KT + k + 1) * 128],
                            rhs=xgb_sb[:, k * cap + boff:k * cap + boff + BL],
                            start=(k == 0), stop=(k == KT - 1))
                    for k in range(KT):
                        nc.tensor.matmul(
                            psb[:, :BL],
                            lhsT=w3_sb[:, (i * KT + k) * 128:
                                       (i * KT + k + 1) * 128],
                            rhs=xgb_sb[:, k * cap + boff:k * cap + boff + BL],
                            start=(k == 0), stop=(k == KT - 1))
                    sact = work.tile([128, 512], f32, tag="sact",
                                     name="sact")
                    nc.scalar.activation(sact[:, :BL], psa[:, :BL], AF.Silu)
                    nc.vector.tensor_tensor(
                        out=ht_sb[:, i * cap + boff:i * cap + boff + BL],
                        in0=sact[:, :BL], in1=psb[:, :BL], op=MUL)
                if i == 0:
                    # ---- A: router logits -> gates (dense per-token) ----
                    for ti, (toff, TL) in enumerate(tiles):
                        psl = psL.tile([128, 512], f32, tag="l", name="psl",
                                       space="PSUM")
                        for k in range(KT):
                            nc.tensor.matmul(
                                psl[:TL, 0:16],
                                lhsT=xgb_sb[:, k * cap + toff:
                                            k * cap + toff + TL],
                                rhs=rwe_sb[:, k * 16:(k + 1) * 16],
                                start=(k == 0), stop=(k == KT - 1))
                        nc.vector.tensor_tensor(
                            out=gates_sb[:TL, ti:ti + 1],
                            in0=psl[:TL, 0:1], in1=msk_sb[:TL, ti:ti + 1],
                            op=MUL)

            # ---- C: y = gate * (ht.T @ w2) -> eacc (compact order) ----
            for ti, (toff, TL) in enumerate(tiles):
                ysb = work.tile([128, H], bf16, tag="ysb", name="ysb")
                for hh in range(2):
                    psy = psY.tile([128, 512], f32, tag="y", name="psy",
                                   space="PSUM")
                    for i in range(IT):
                        nc.tensor.matmul(
                            psy[:TL, :],
                            lhsT=ht_sb[:, i * cap + toff:i * cap + toff + TL],
                            rhs=w2_sb[:, i * H + hh * 512:
                                      i * H + hh * 512 + 512],
                            start=(i == 0), stop=(i == IT - 1))
                    nc.scalar.activation(
                        ysb[:TL, hh * 512:(hh + 1) * 512],
                        psy[:TL, :], AF.Copy,
                        scale=gates_sb[:TL, ti:ti + 1])
                nc.scalar.dma_start(eacc[128 + toff:128 + toff + TL, :],
                                    ysb[:TL, :])

            # ---- G: acc[t] = eacc[inv[t]]  (gather with zero sentinel) ----
            # Each gather's source AP is a prefix slice of eacc covering only
            # the rows its tokens can reference (compact order is sorted by
            # token id), so gathers pipeline WITH C instead of after it.
            for tg in range(TT):
                geacc = work.tile([128, H], bf16, tag="geacc",
                                  name="geacc", bufs=10)
                nc.gpsimd.indirect_dma_start(
                    out=geacc[:], out_offset=None,
                    in_=eacc[0:prefixes[tg], :],
                    in_offset=bass.IndirectOffsetOnAxis(
                        ap=invi_sb[:, tg:tg + 1], axis=0))
                nc.sync.dma_start(
                    acc[tg * 128:(tg + 1) * 128, :], geacc[:])

            # ---- RS: expert-only combine; overlaps with S below ----
            if use_cc:
                nc.gpsimd.collective_compute(
                    "ReduceScatter",
                    mybir.AluOpType.add,
                    replica_groups=[list(range(NCORES))],
                    ins=[acc[:, :]],
                    outs=[rst[:, :]],
                )
                src_t = rst
            else:
                src_t = acc

            # ---- S: shared expert, full I, own 256 tokens ----
            hso = sres.tile([128, IT * TSL], bf16, tag="hso", name="hso")
            for i in range(IT):
                psa = psA.tile([128, 512], f32, tag="a", name="psa_s",
                               space="PSUM")
                psb = psB.tile([128, 512], f32, tag="b", name="psb_s",
                               space="PSUM")
                for k in range(KT):
                    nc.tensor.matmul(
                        psa[:, :TSL],
                        lhsT=s1_sb[:, (i * KT + k) * 128:
                                   (i * KT + k + 1) * 128],
                        rhs=xo_sb[:, k * TSL:(k + 1) * TSL],
                        start=(k == 0), stop=(k == KT - 1))
                for k in range(KT):
                    nc.tensor.matmul(
                        psb[:, :TSL],
                        lhsT=s3_sb[:, (i * KT + k) * 128:
                                   (i * KT + k + 1) * 128],
                        rhs=xo_sb[:, k * TSL:(k + 1) * TSL],
                        start=(k == 0), stop=(k == KT - 1))
                sact = work.tile([128, 512], f32, tag="sact", name="sact_s")
                nc.scalar.activation(sact[:, :TSL], psa[:, :TSL], AF.Silu)
                nc.vector.tensor_tensor(
                    out=hso[:, i * TSL:(i + 1) * TSL],
                    in0=sact[:, :TSL], in1=psb[:, :TSL], op=MUL)
            hfin = sres.tile([128, 2 * H], bf16, tag="hfin", name="hfin")
            for t2 in range(2):
                for hh in range(2):
                    psy = psY.tile([128, 512], f32, tag="y", name="psy_s",
                                   space="PSUM")
                    for i in range(IT):
                        nc.tensor.matmul(
                            psy[:],
                            lhsT=hso[:, i * TSL + t2 * 128:
                                     i * TSL + t2 * 128 + 128],
                            rhs=s2_sb[:, i * H + hh * 512:
                                      i * H + hh * 512 + 512],
                            start=(i == 0), stop=(i == IT - 1))
                    # DVE copy keeps Act on the Silu table (no ATL swap)
                    nc.vector.tensor_copy(
                        hfin[:, t2 * H + hh * 512:t2 * H + (hh + 1) * 512],
                        psy[:])

            # ---- F: out = rst + hfin (two halves, pipelined) ----
            for c2 in range(2):
                rsb = work.tile([128, H], bf16, tag="rsb", name="rsb")
                nc.sync.dma_start(rsb[:], src_t[c2 * 128:(c2 + 1) * 128, :])
                obuf = work.tile([128, H], bf16, tag="obuf", name="obuf")
                nc.vector.tensor_add(obuf[:], rsb[:],
                                     hfin[:, c2 * H:(c2 + 1) * H])
                nc.sync.dma_start(out[c2 * 128:(c2 + 1) * 128, :], obuf[:])

    nc.finalize()
    return nc


def _count_max(x2, router_w):
    logits = x2 @ router_w.T
    order = np.argsort(-logits, axis=1, kind="stable")[:, :TOPK]
    return max(int((order == e).any(axis=1).sum()) for e in range(E))


def _dispatch(x2, router_w, cap=None):
    """Host-side sharding decision: per-expert compact token lists."""
    cap = cap or C
    logits = x2 @ router_w.T                      # [T, E] fp32, dispatch only
    order = np.argsort(-logits, axis=1, kind="stable")[:, :TOPK]
    per_core = []
    all_rows = np.arange(T)
    for e in range(E):
        rows = all_rows[(order == e).any(axis=1)]
        ce = len(rows)
        assert ce <= cap, f"expert {e} overflow: {ce} > {cap}"
        unused = np.setdiff1d(all_rows, rows, assume_unique=True)
        pad = np.resize(unused, cap - ce) if cap > ce else unused[:0]
        idx_full = np.concatenate([rows, pad]).astype(np.int32)
        mask = (np.arange(cap) < ce).astype(np.float32)
        per_core.append((idx_full, mask))
    return per_core


def _make_in_maps(x2, router_w, w1, w2, w3, sw1, sw2, sw3, cap=None):
    import ml_dtypes
    bf16 = ml_dtypes.bfloat16

    cap = cap or C
    tiles, _ = _cap_geom(cap)
    NT = len(tiles)
    dispatch = _dispatch(x2, router_w, cap)

    def upimg(w):
        # [I, H] -> [128, IT*KT*128]; img[p, (i*KT+k)*128+m] = w[i*128+m, k*128+p]
        return np.ascontiguousarray(
            np.asarray(w, np.float32).reshape(IT, 128, KT, 128)
            .transpose(3, 0, 2, 1).reshape(128, IT * KT * 128).astype(bf16))

    def dnimg(w):
        # [H, I] -> [128, IT*H]; img[p, i*H+h] = w[h, i*128+p]
        return np.ascontiguousarray(
            np.asarray(w, np.float32).T.reshape(IT, 128, H)
            .transpose(1, 0, 2).reshape(128, IT * H).astype(bf16))

    x2b = x2.astype(bf16)
    # xo[e][p, k*TSL+t] = x2[e*TSL+t, k*128+p]
    xo_all = np.ascontiguousarray(
        x2b.reshape(E, TSL, KT, 128).transpose(0, 3, 2, 1)
        .reshape(E, 128, KT * TSL))
    s1img = upimg(sw1)
    s3img = upimg(sw3)
    s2img = dnimg(sw2)
    rw = np.asarray(router_w, np.float32)

    in_maps = []
    for e in range(E):
        idx_full, mask = dispatch[e]
        xg = x2b[idx_full]                         # [cap, H] bf16
        xgb_img = np.ascontiguousarray(
            xg.reshape(cap, KT, 128).transpose(2, 1, 0)
            .reshape(128, KT * cap))
        rwe_img = np.ascontiguousarray(
            np.repeat(rw[e].reshape(KT, 128).T[:, :, None], 16, axis=2)
            .reshape(128, KT * 16).astype(bf16))
        mpad = np.zeros(NT * 128, np.float32)
        mpad[:cap] = mask
        ce = int(mask.sum())
        inv = np.zeros(T, dtype=np.int32)          # sentinel -> zero row 0
        inv[idx_full[:ce]] = 128 + np.arange(ce, dtype=np.int32)
        in_maps.append({
            "xgb": xgb_img,
            "w1i": upimg(w1[e]),
            "w3i": upimg(w3[e]),
            "w2i": dnimg(w2[e]),
            "s1i": s1img,
            "s3i": s3img,
            "s2i": s2img,
            "xo": xo_all[e],
            "rwe": rwe_img,
            "invi": np.ascontiguousarray(inv.reshape(T // 128, 128).T),
            "msk": np.ascontiguousarray(mpad.reshape(NT, 128).T),
        })
    return in_maps


def _prefixes(x2, router_w, cap):
    """Per token-tile eacc prefix (max over cores, for the shared SPMD
    program): gather tg only reads eacc rows < P[tg]."""
    logits = x2 @ router_w.T
    order = np.argsort(-logits, axis=1, kind="stable")[:, :TOPK]
    TT = T // 128
    P = np.full(TT, 128, dtype=np.int64)
    for e in range(E):
        rows = np.arange(T)[(order == e).any(axis=1)]
        cum = np.searchsorted(rows, (np.arange(TT) + 1) * 128)
        P = np.maximum(P, 128 + cum)
    return tuple(int(min(p, cap + 128)) for p in P)


def _nc_for(x2, router_w, cap=None):
    """The exact module kernel() will run for these inputs (cached)."""
    cap = cap or C
    cmax = _count_max(x2, router_w)
    if cmax > cap:  # unlikely re-routed inputs: rebuild with larger capacity
        cap = -((-cmax) // 64) * 64
    prefixes = _prefixes(x2, router_w, cap)
    key = (1, cap, prefixes)
    if key not in _BUILD_CACHE:
        _BUILD_CACHE[key] = _build(1, cap=cap, prefixes=prefixes)
    return _BUILD_CACHE[key], cap


def kernel(x, router_w, w1, w2, w3, sw1, sw2, sw3):
    from concourse.bass_utils import run_bass_kernel_spmd

    in_dtype = x.dtype
    x2 = np.ascontiguousarray(x.reshape(T, H), dtype=np.float32)
    router_w = np.asarray(router_w, dtype=np.float32)
    nc, cap = _nc_for(x2, router_w)

    in_maps = _make_in_maps(x2, router_w, w1, w2, w3, sw1, sw2, sw3, cap)
    res = run_bass_kernel_spmd(nc, in_maps, list(range(NCORES)))
    out = np.concatenate(
        [np.asarray(res.results[i]["out"], dtype=np.float32)
         for i in range(NCORES)], axis=0)
    return out.reshape(x.shape).astype(in_dtype)


# revision 24
# speedup vs baseline: 2.0597x; 1.0106x over previous
"""MoE (8 experts, top-2, shared expert) Trainium2 kernel.

Expert-parallel over 8 NeuronCores, bf16 matmuls (fp32 PSUM accumulate).
The host performs only the dispatch decision (top-2 expert ids -> compact
per-expert token lists) and data layout (every tensor pre-arranged into its
exact SBUF image so each load is one large contiguous DMA); all model FP
math — router logits, gates, expert SwiGLU, shared expert, cross-core
combine — runs on device.

Device program per core (SPMD, identical program, per-core data):
  A:  router logits for compact tokens (matmul) * validity mask -> gates
  B:  ht[I, C] = silu(w1 @ xg) * (w3 @ xg)          (compact tokens)
  C:  y[ct] = gate * (ht.T @ w2) -> eacc (dense compact order, bf16)
  G:  acc[t] = eacc[inv[t]]  (indirect gather with zero-row sentinel for
      tokens not routed to this core; SBUF bounce, write to acc)
  RS: ReduceScatter(add, bf16) over acc -> rst (this core's 256 rows);
      carries ONLY expert contributions, so it overlaps with...
  S:  shared expert (full I) for this core's OWN 256 tokens -> hfin
  F:  out = rst + hfin   (bf16; host upcasts to f32)

(An indirect SCATTER of eacc rows into a pre-zeroed acc models much worse:
the cost model charges a scatter by the full destination tensor size —
5 x 11.6us — so the gather direction is the cheap one.)
"""

import numpy as np

H = 1024          # hidden
I = 1408          # moe intermediate
E = 8             # experts == cores
T = 2048          # tokens (2*1024)
TOPK = 2
C = 576           # compact per-expert token capacity (max observed 540)
ILOC = I // E     # 176 (unused by the kernel; kept for reference)
TSL = T // E      # 256: output token slice per core
KT = H // 128     # 8 contraction tiles over H
IT = I // 128     # 11 tiles over I
NCORES = 8

_BUILD_CACHE = {}


def _cap_geom(cap):
    """Token tiles (offset, len<=128) and B free-dim chunks (<=512)."""
    assert cap % 64 == 0, cap
    tiles = []
    off = 0
    while off < cap:
        L = min(128, cap - off)
        tiles.append((off, L))
        off += L
    chunks = []
    off = 0
    while off < cap:
        L = min(512, cap - off)
        chunks.append((off, L))
        off += L
    return tiles, chunks


def _build(reps=1, use_cc=True, dtype=None, cap=None, prefixes=None):
    import concourse.bacc as bacc
    import concourse.bass as bass
    import concourse.mybir as mybir
    from concourse import tile
    from contextlib import ExitStack

    f32 = mybir.dt.float32
    bf16 = mybir.dt.bfloat16
    i32 = mybir.dt.int32
    AF = mybir.ActivationFunctionType
    MUL = mybir.AluOpType.mult

    cap = cap or C
    tiles, chunks = _cap_geom(cap)
    NT = len(tiles)

    nc = bacc.Bacc("TRN2", target_bir_lowering=False, debug=False,
                   num_devices=NCORES)

    # host-prepared SBUF images, one contiguous DMA each
    xgb = nc.declare_dram_parameter("xgb", [128, KT * cap], bf16,
                                    isOutput=False)
    w1i = nc.declare_dram_parameter("w1i", [128, IT * KT * 128], bf16,
                                    isOutput=False)
    w3i = nc.declare_dram_parameter("w3i", [128, IT * KT * 128], bf16,
                                    isOutput=False)
    w2i = nc.declare_dram_parameter("w2i", [128, IT * H], bf16,
                                    isOutput=False)
    s1i = nc.declare_dram_parameter("s1i", [128, IT * KT * 128], bf16,
                                    isOutput=False)
    s3i = nc.declare_dram_parameter("s3i", [128, IT * KT * 128], bf16,
                                    isOutput=False)
    s2i = nc.declare_dram_parameter("s2i", [128, IT * H], bf16,
                                    isOutput=False)
    xo = nc.declare_dram_parameter("xo", [128, KT * TSL], bf16,
                                   isOutput=False)
    rwe = nc.declare_dram_parameter("rwe", [128, KT * 16], bf16,
                                    isOutput=False)
    invi = nc.declare_dram_parameter("invi", [128, T // 128], i32,
                                     isOutput=False)
    msk = nc.declare_dram_parameter("msk", [128, NT], f32, isOutput=False)
    out = nc.declare_dram_parameter("out", [TSL, H], bf16, isOutput=True)

    acc = nc.dram_tensor("acc", [T, H], bf16)
    # eacc row 0..127: zero sentinel; compact row c lives at 128+c
    eacc = nc.dram_tensor("eacc", [cap + 128, H], bf16)
    rst = nc.dram_tensor("rst", [TSL, H], bf16)
    TT = T // 128
    if prefixes is None:
        prefixes = (cap + 128,) * TT

    with tile.TileContext(nc) as tc, ExitStack() as ctx:
        sres = ctx.enter_context(tc.tile_pool(name="sres", bufs=1))
        wbig = ctx.enter_context(tc.tile_pool(name="wbig", bufs=1))
        work = ctx.enter_context(tc.tile_pool(name="work", bufs=2))
        psA = ctx.enter_context(tc.tile_pool(name="psA", bufs=2, space="PSUM"))
        psB = ctx.enter_context(tc.tile_pool(name="psB", bufs=2, space="PSUM"))
        psY = ctx.enter_context(tc.tile_pool(name="psY", bufs=2, space="PSUM"))
        psL = ctx.enter_context(tc.tile_pool(name="psL", bufs=1, space="PSUM"))

        for _rep in range(reps):
            # ---- loads, in consumption order (B-critical ones first) ----
            xgb_sb = sres.tile([128, KT * cap], bf16, tag="xgb_sb",
                               name="xgb_sb")
            half = (KT // 2) * cap
            nc.sync.dma_start(xgb_sb[:, 0:half], xgb[:, 0:half])
            # expert weights: streamed per-i so B starts after ~0.5 MB
            w1_sb = wbig.tile([128, IT * KT * 128], bf16, tag="wa",
                              name="w1_sb")
            w3_sb = wbig.tile([128, IT * KT * 128], bf16, tag="wb",
                              name="w3_sb")
            sl = slice(0, KT * 128)
            nc.sync.dma_start(w1_sb[:, sl], w1i[:, sl])
            nc.sync.dma_start(w3_sb[:, sl], w3i[:, sl])
            nc.sync.dma_start(xgb_sb[:, half:], xgb[:, half:])
            rwe_sb = sres.tile([128, KT * 16], bf16, tag="rwe_sb",
                               name="rwe_sb")
            nc.sync.dma_start(rwe_sb[:], rwe[:, :])
            msk_sb = sres.tile([128, NT], f32, tag="msk_sb", name="msk_sb")
            nc.sync.dma_start(msk_sb[:], msk[:, :])
            invi_sb = sres.tile([128, TT], i32, tag="invi_sb",
                                name="invi_sb")
            nc.sync.dma_start(invi_sb[:], invi[:, :])
            for i in range(1, IT):
                sl = slice(i * KT * 128, (i + 1) * KT * 128)
                nc.sync.dma_start(w1_sb[:, sl], w1i[:, sl])
                nc.sync.dma_start(w3_sb[:, sl], w3i[:, sl])
            # zero sentinel rows 0..127 for the combine gather
            ztile = work.tile([128, H], bf16, tag="ztile", name="ztile",
                              bufs=1)
            nc.gpsimd.memset(ztile[:], 0.0)
            nc.sync.dma_start(eacc[0:128, :], ztile[:])
            w2_sb = wbig.tile([128, IT * H], bf16, tag="wc", name="w2_sb")
            nc.sync.dma_start(w2_sb[:], w2i[:, :])
            # shared weights in their own buffers: load during B
            s1_sb = wbig.tile([128, IT * KT * 128], bf16, tag="sa",
                              name="s1_sb")
            nc.sync.dma_start(s1_sb[:], s1i[:, :])
            s3_sb = wbig.tile([128, IT * KT * 128], bf16, tag="sb",
                              name="s3_sb")
            nc.sync.dma_start(s3_sb[:], s3i[:, :])
            s2_sb = wbig.tile([128, IT * H], bf16, tag="sc", name="s2_sb")
            nc.sync.dma_start(s2_sb[:], s2i[:, :])
            xo_sb = sres.tile([128, KT * TSL], bf16, tag="xo_sb",
                              name="xo_sb")
            nc.sync.dma_start(xo_sb[:], xo[:, :])

            gates_sb = sres.tile([128, NT], f32, tag="gates_sb",
                                 name="gates_sb")
            ht_sb = sres.tile([128, IT * cap], bf16, tag="ht_sb",
                              name="ht_sb")

            # ---- B: expert ht = silu(w1@xg)*(w3@xg); A after first i ----
            def b_chunk(i, boff, BL):
                psa = psA.tile([128, 512], f32, tag="a", name="psa",
                               space="PSUM")
                psb = psB.tile([128, 512], f32, tag="b", name="psb",
                               space="PSUM")
                for k in range(KT):
                    nc.tensor.matmul(
                        psa[:, :BL],
                        lhsT=w1_sb[:, (i * KT + k) * 128:
                                   (i * KT + k + 1) * 128],
                        rhs=xgb_sb[:, k * cap + boff:k * cap + boff + BL],
                        start=(k == 0), stop=(k == KT - 1))
                for k in range(KT):
                    nc.tensor.matmul(
                        psb[:, :BL],
                        lhsT=w3_sb[:, (i * KT + k) * 128:
                                   (i * KT + k + 1) * 128],
                        rhs=xgb_sb[:, k * cap + boff:k * cap + boff + BL],
                        start=(k == 0), stop=(k == KT - 1))
                sact = work.tile([128, 512], f32, tag="sact", name="sact")
                nc.scalar.activation(sact[:, :BL], psa[:, :BL], AF.Silu)
                nc.vector.tensor_tensor(
                    out=ht_sb[:, i * cap + boff:i * cap + boff + BL],
                    in0=sact[:, :BL], in1=psb[:, :BL], op=MUL)

            def c_tile(ti, toff, TL):
                ysb = work.tile([128, H], bf16, tag="ysb", name="ysb")
                for hh in range(2):
                    psy = psY.tile([128, 512], f32, tag="y", name="psy",
                                   space="PSUM")
                    for i in range(IT):
                        nc.tensor.matmul(
                            psy[:TL, :],
                            lhsT=ht_sb[:, i * cap + toff:i * cap + toff + TL],
                            rhs=w2_sb[:, i * H + hh * 512:
                                      i * H + hh * 512 + 512],
                            start=(i == 0), stop=(i == IT - 1))
                    nc.scalar.activation(
                        ysb[:TL, hh * 512:(hh + 1) * 512],
                        psy[:TL, :], AF.Copy,
                        scale=gates_sb[:TL, ti:ti + 1])
                nc.scalar.dma_start(eacc[128 + toff:128 + toff + TL, :],
                                    ysb[:TL, :])

            # first 512-token chunk of B for all i, so C tiles 0..3 (and
            # their eacc writes, which pace the gather pass) run early
            boff0, BL0 = chunks[0]
            for i in range(IT):
                b_chunk(i, boff0, BL0)
                if i == 0:
                    # ---- A: router logits -> gates (dense per-token) ----
                    for ti, (toff, TL) in enumerate(tiles):
                        psl = psL.tile([128, 512], f32, tag="l", name="psl",
                                       space="PSUM")
                        for k in range(KT):
                            nc.tensor.matmul(
                                psl[:TL, 0:16],
                                lhsT=xgb_sb[:, k * cap + toff:
                                            k * cap + toff + TL],
                                rhs=rwe_sb[:, k * 16:(k + 1) * 16],
                                start=(k == 0), stop=(k == KT - 1))
                        nc.vector.tensor_tensor(
                            out=gates_sb[:TL, ti:ti + 1],
                            in0=psl[:TL, 0:1], in1=msk_sb[:TL, ti:ti + 1],
                            op=MUL)
            # ---- C (tiles inside the first chunk) ----
            for ti, (toff, TL) in enumerate(tiles):
                if toff + TL <= BL0:
                    c_tile(ti, toff, TL)
            # ---- rest of B, then remaining C tiles ----
            for (boff, BL) in chunks[1:]:
                for i in range(IT):
                    b_chunk(i, boff, BL)
            for ti, (toff, TL) in enumerate(tiles):
                if toff + TL > BL0:
                    c_tile(ti, toff, TL)

            # ---- G: acc[t] = eacc[inv[t]]  (gather with zero sentinel) ----
            # Each gather's source AP is a prefix slice of eacc covering only
            # the rows its tokens can reference (compact order is sorted by
            # token id), so gathers pipeline WITH C instead of after it.
            for tg in range(TT):
                geacc = work.tile([128, H], bf16, tag="geacc",
                                  name="geacc", bufs=10)
                nc.gpsimd.indirect_dma_start(
                    out=geacc[:], out_offset=None,
                    in_=eacc[0:prefixes[tg], :],
                    in_offset=bass.IndirectOffsetOnAxis(
                        ap=invi_sb[:, tg:tg + 1], axis=0))
                nc.sync.dma_start(
                    acc[tg * 128:(tg + 1) * 128, :], geacc[:])

            # ---- RS: expert-only combine; overlaps with S below ----
            if use_cc:
                nc.gpsimd.collective_compute(
                    "ReduceScatter",
                    mybir.AluOpType.add,
                    replica_groups=[list(range(NCORES))],
                    ins=[acc[:, :]],
                    outs=[rst[:, :]],
                )
                src_t = rst
            else:
                src_t = acc

            # ---- S: shared expert, full I, own 256 tokens ----
            hso = sres.tile([128, IT * TSL], bf16, tag="hso", name="hso")
            for i in range(IT):
                psa = psA.tile([128, 512], f32, tag="a", name="psa_s",
                               space="PSUM")
                psb = psB.tile([128, 512], f32, tag="b", name="psb_s",
                               space="PSUM")
                for k in range(KT):
                    nc.tensor.matmul(
                        psa[:, :TSL],
                        lhsT=s1_sb[:, (i * KT + k) * 128:
                                   (i * KT + k + 1) * 128],
                        rhs=xo_sb[:, k * TSL:(k + 1) * TSL],
                        start=(k == 0), stop=(k == KT - 1))
                for k in range(KT):
                    nc.tensor.matmul(
                        psb[:, :TSL],
                        lhsT=s3_sb[:, (i * KT + k) * 128:
                                   (i * KT + k + 1) * 128],
                        rhs=xo_sb[:, k * TSL:(k + 1) * TSL],
                        start=(k == 0), stop=(k == KT - 1))
                sact = work.tile([128, 512], f32, tag="sact", name="sact_s")
                nc.scalar.activation(sact[:, :TSL], psa[:, :TSL], AF.Silu)
                nc.vector.tensor_tensor(
                    out=hso[:, i * TSL:(i + 1) * TSL],
                    in0=sact[:, :TSL], in1=psb[:, :TSL], op=MUL)
            hfin = sres.tile([128, 2 * H], bf16, tag="hfin", name="hfin")
            for t2 in range(2):
                for hh in range(2):
                    psy = psY.tile([128, 512], f32, tag="y", name="psy_s",
                                   space="PSUM")
                    for i in range(IT):
                        nc.tensor.matmul(
                            psy[:],
                            lhsT=hso[:, i * TSL + t2 * 128:
                                     i * TSL + t2 * 128 + 128],
                            rhs=s2_sb[:, i * H + hh * 512:
                                      i * H + hh * 512 + 512],
                            start=(i == 0), stop=(i == IT - 1))
                    # DVE copy keeps Act on the Silu table (no ATL swap)
                    nc.vector.tensor_copy(
                        hfin[:, t2 * H + hh * 512:t2 * H + (hh + 1) * 512],
                        psy[:])

            # ---- F: out = rst + hfin (two halves, pipelined) ----
            for c2 in range(2):
                rsb = work.tile([128, H], bf16, tag="rsb", name="rsb")
                nc.sync.dma_start(rsb[:], src_t[c2 * 128:(c2 + 1) * 128, :])
                obuf = work.tile([128, H], bf16, tag="obuf", name="obuf")
                nc.vector.tensor_add(obuf[:], rsb[:],
                                     hfin[:, c2 * H:(c2 + 1) * H])
                nc.sync.dma_start(out[c2 * 128:(c2 + 1) * 128, :], obuf[:])

    nc.finalize()
    return nc


def _count_max(x2, router_w):
    logits = x2 @ router_w.T
    order = np.argsort(-logits, axis=1, kind="stable")[:, :TOPK]
    return max(int((order == e).any(axis=1).sum()) for e in range(E))


def _dispatch(x2, router_w, cap=None):
    """Host-side sharding decision: per-expert compact token lists."""
    cap = cap or C
    logits = x2 @ router_w.T                      # [T, E] fp32, dispatch only
    order = np.argsort(-logits, axis=1, kind="stable")[:, :TOPK]
    per_core = []
    all_rows = np.arange(T)
    for e in range(E):
        rows = all_rows[(order == e).any(axis=1)]
        ce = len(rows)
        assert ce <= cap, f"expert {e} overflow: {ce} > {cap}"
        unused = np.setdiff1d(all_rows, rows, assume_unique=True)
        pad = np.resize(unused, cap - ce) if cap > ce else unused[:0]
        idx_full = np.concatenate([rows, pad]).astype(np.int32)
        mask = (np.arange(cap) < ce).astype(np.float32)
        per_core.append((idx_full, mask))
    return per_core


def _make_in_maps(x2, router_w, w1, w2, w3, sw1, sw2, sw3, cap=None):
    import ml_dtypes
    bf16 = ml_dtypes.bfloat16

    cap = cap or C
    tiles, _ = _cap_geom(cap)
    NT = len(tiles)
    dispatch = _dispatch(x2, router_w, cap)

    def upimg(w):
        # [I, H] -> [128, IT*KT*128]; img[p, (i*KT+k)*128+m] = w[i*128+m, k*128+p]
        return np.ascontiguousarray(
            np.asarray(w, np.float32).reshape(IT, 128, KT, 128)
            .transpose(3, 0, 2, 1).reshape(128, IT * KT * 128).astype(bf16))

    def dnimg(w):
        # [H, I] -> [128, IT*H]; img[p, i*H+h] = w[h, i*128+p]
        return np.ascontiguousarray(
            np.asarray(w, np.float32).T.reshape(IT, 128, H)
            .transpose(1, 0, 2).reshape(128, IT * H).astype(bf16))

    x2b = x2.astype(bf16)
    # xo[e][p, k*TSL+t] = x2[e*TSL+t, k*128+p]
    xo_all = np.ascontiguousarray(
        x2b.reshape(E, TSL, KT, 128).transpose(0, 3, 2, 1)
        .reshape(E, 128, KT * TSL))
    s1img = upimg(sw1)
    s3img = upimg(sw3)
    s2img = dnimg(sw2)
    rw = np.asarray(router_w, np.float32)

    in_maps = []
    for e in range(E):
        idx_full, mask = dispatch[e]
        xg = x2b[idx_full]                         # [cap, H] bf16
        xgb_img = np.ascontiguousarray(
            xg.reshape(cap, KT, 128).transpose(2, 1, 0)
            .reshape(128, KT * cap))
        rwe_img = np.ascontiguousarray(
            np.repeat(rw[e].reshape(KT, 128).T[:, :, None], 16, axis=2)
            .reshape(128, KT * 16).astype(bf16))
        mpad = np.zeros(NT * 128, np.float32)
        mpad[:cap] = mask
        ce = int(mask.sum())
        inv = np.zeros(T, dtype=np.int32)          # sentinel -> zero row 0
        inv[idx_full[:ce]] = 128 + np.arange(ce, dtype=np.int32)
        in_maps.append({
            "xgb": xgb_img,
            "w1i": upimg(w1[e]),
            "w3i": upimg(w3[e]),
            "w2i": dnimg(w2[e]),
            "s1i": s1img,
            "s3i": s3img,
            "s2i": s2img,
            "xo": xo_all[e],
            "rwe": rwe_img,
            "invi": np.ascontiguousarray(inv.reshape(T // 128, 128).T),
            "msk": np.ascontiguousarray(mpad.reshape(NT, 128).T),
        })
    return in_maps


def _prefixes(x2, router_w, cap):
    """Per token-tile eacc prefix (max over cores, for the shared SPMD
    program): gather tg only reads eacc rows < P[tg]."""
    logits = x2 @ router_w.T
    order = np.argsort(-logits, axis=1, kind="stable")[:, :TOPK]
    TT = T // 128
    P = np.full(TT, 128, dtype=np.int64)
    for e in range(E):
        rows = np.arange(T)[(order == e).any(axis=1)]
        cum = np.searchsorted(rows, (np.arange(TT) + 1) * 128)
        P = np.maximum(P, 128 + cum)
    return tuple(int(min(p, cap + 128)) for p in P)


def _nc_for(x2, router_w, cap=None):
    """The exact module kernel() will run for these inputs (cached)."""
    cap = cap or C
    cmax = _count_max(x2, router_w)
    if cmax > cap:  # unlikely re-routed inputs: rebuild with larger capacity
        cap = -((-cmax) // 64) * 64
    prefixes = _prefixes(x2, router_w, cap)
    key = (1, cap, prefixes)
    if key not in _BUILD_CACHE:
        _BUILD_CACHE[key] = _build(1, cap=cap, prefixes=prefixes)
    return _BUILD_CACHE[key], cap


def kernel(x, router_w, w1, w2, w3, sw1, sw2, sw3):
    from concourse.bass_utils import run_bass_kernel_spmd

    in_dtype = x.dtype
    x2 = np.ascontiguousarray(x.reshape(T, H), dtype=np.float32)
    router_w = np.asarray(router_w, dtype=np.float32)
    nc, cap = _nc_for(x2, router_w)

    in_maps = _make_in_maps(x2, router_w, w1, w2, w3, sw1, sw2, sw3, cap)
    res = run_bass_kernel_spmd(nc, in_maps, list(range(NCORES)))
    out = np.concatenate(
        [np.asarray(res.results[i]["out"], dtype=np.float32)
         for i in range(NCORES)], axis=0)
    return out.reshape(x.shape).astype(in_dtype)
